# revision 23
# baseline (speedup 1.0000x reference)
"""Tensor-parallel GQA attention prefill (B=1, T=2048, D=4096, 32 q-heads /
8 kv-heads) for 8 Trainium2 NeuronCores.

Sharding: head-parallel.  Core c owns q-heads [4c, 4c+4) and kv-head c.
  phase 1: Q/K/V projections in transposed layout (head-dim on partitions),
           RoPE applied via a rotation-matmul + two table multiplies.
  phase 2: per-head attention with scores held transposed (tk on
           partitions); softmax denominators come from a ones-matmul;
           fully-masked tiles are skipped (host inspects the mask tensor).
  phase 3: output projection sharded over the CONTRACTION dim: each core
           multiplies its local attention slice (512 rows) against its
           512-row slice of wo, producing partial sums for ALL 4096 output
           dims; a per-tq-chunk ReduceScatter (bf16) then both sums the
           partials and hands each core its own 512 output rows.
  phase 4: bias add + writeback of the scattered result.

Matmul operands are bf16 (fp32 accumulation in PSUM); measured end-to-end
error vs the fp32 reference is ~4e-3 relative.

NOTE: faithful to the reference "bug" -- the q projection uses wo_w/wo_b.
"""

import numpy as np
import ml_dtypes

import bass_rust
import concourse.bass as bass
import concourse.mybir as mybir
import concourse.tile as tile
from concourse.bass_utils import run_bass_kernel_spmd
from concourse.masks import make_identity

# problem constants (self-contained; do not read spec.json)
DIM = 4096
NH = 32
NKV = 8
HD = 128
T = 2048
NCORE = 8
HPC = NH // NCORE      # 4 q heads per core
JPC = HPC * HD         # 512 output columns per core
P = 128
NT = T // 512          # 4 free-dim chunks of 512
NKC = DIM // P         # 32 contraction chunks in the projections
TKC = T // P           # 16 tk chunks in attention
SCALE = 1.0 / float(np.sqrt(HD))

F32 = mybir.dt.float32
BF16 = mybir.dt.bfloat16

# mask tile classification
MSK_SKIP, MSK_ZERO, MSK_ADD = 0, 1, 2


def legalize_waits(nc, max_waits=1):
    """Hoist excess on_wait conditions onto preceding nop instructions.

    This walrus build rejects instructions carrying more than a couple of
    sync-wait commands; engines execute their queue in order, so a nop that
    waits immediately before the real instruction is equivalent.
    """
    n_new = 0
    for f in nc.m.functions:
        for bb in f.blocks:
            insts = bb.instructions
            new = []
            for ins in list(insts):
                si = ins.sync_info
                waits = list(si.on_wait) if si is not None and si.on_wait else []
                if len(waits) > max_waits:
                    hoist = waits[:-max_waits]
                    keep = waits[-max_waits:]
                    for j in range(0, len(hoist), max_waits):
                        chunk = hoist[j:j + max_waits]
                        nop = mybir.InstNoOp(
                            name=f"{ins.name}_hw{j}",
                            engine=ins.engine,
                            sync_info=bass_rust.SyncInfo(
                                on_wait=chunk, on_update=[]),
                        )
                        new.append(nop)
                        n_new += 1
                    ins.sync_info = bass_rust.SyncInfo(
                        on_wait=keep,
                        on_update=list(si.on_update) if si.on_update else [])
                new.append(ins)
            insts.clear()
            insts.extend(new)
    return n_new


def _classify_mask(mask):
    """Per (tk-chunk, tq-chunk-of-512) classification of the additive mask.

    Returns (klass, col0) where col0[k, c] is the first tq column (multiple
    of 128) of the chunk that is not fully masked -- matmuls/exp for the
    columns before it are skipped (their softmax weights are exactly 0).
    """
    klass = np.empty((TKC, NT), dtype=np.int32)
    col0 = np.zeros((TKC, NT), dtype=np.int32)
    for k in range(TKC):
        for c in range(NT):
            blk = mask[c * 512:(c + 1) * 512, k * P:(k + 1) * P]
            mx = float(blk.max())
            mn = float(blk.min())
            if mx < -80.0:
                klass[k, c] = MSK_SKIP
                continue
            if mx == 0.0 and mn == 0.0:
                klass[k, c] = MSK_ZERO
            else:
                klass[k, c] = MSK_ADD
            # leading fully-masked tq columns, rounded down to 128
            colmax = blk.max(axis=1)          # per-tq-row max over this tile
            nz = np.nonzero(colmax >= -80.0)[0]
            first = int(nz[0]) if len(nz) else 0
            first = (first // P) * P
            # only safe to skip if every column before `first` is fully masked
            if first > 0 and float(blk[:first].max()) < -80.0:
                col0[k, c] = first
    # never allow a fully-empty (all-skip) tq chunk; keep one tile live
    for c in range(NT):
        if all(klass[k, c] == MSK_SKIP for k in range(TKC)):
            klass[min(c * 4, TKC - 1), c] = MSK_ADD
    return klass, col0


def _build_module(klass, col0, phases=(1, 2, 25, 3)):
    nc = bass.Bass()

    # inputs are pre-reblocked on the host so every DMA is contiguous
    xTb = nc.declare_dram_parameter("xTb", [NT, DIM, 512], BF16, isOutput=False)
    woT = nc.declare_dram_parameter("woT", [DIM, JPC], BF16, isOutput=False)
    woR = nc.declare_dram_parameter("woR", [JPC, DIM], BF16, isOutput=False)
    wkT = nc.declare_dram_parameter("wkT", [DIM, HD], BF16, isOutput=False)
    wvT = nc.declare_dram_parameter("wvT", [DIM, HD], BF16, isOutput=False)
    qb = nc.declare_dram_parameter("qb", [P, HPC], F32, isOutput=False)
    qb2 = nc.declare_dram_parameter("qb2", [P, HPC], F32, isOutput=False)
    kb = nc.declare_dram_parameter("kb", [P, 1], F32, isOutput=False)
    vb = nc.declare_dram_parameter("vb", [P, 1], F32, isOutput=False)
    maskTb = nc.declare_dram_parameter("maskTb", [NT, T, 512], BF16,
                                       isOutput=False)
    cost = nc.declare_dram_parameter("cost", [P, T], BF16, isOutput=False)
    sint = nc.declare_dram_parameter("sint", [P, T], BF16, isOutput=False)
    rT = nc.declare_dram_parameter("rT", [P, P], BF16, isOutput=False)
    outTb = nc.declare_dram_parameter("outTb", [NT, JPC, 512], F32,
                                      isOutput=True)

    rs_in = nc.dram_tensor("rs_in", [NT, NCORE * JPC, 512], BF16)
    rs_out = nc.dram_tensor("rs_out", [NT, JPC, 512], BF16)

    with tile.TileContext(nc) as tc:
        with (
            tc.tile_pool(name="wpool", bufs=1) as wpool,
            tc.tile_pool(name="const", bufs=1) as constp,
            tc.tile_pool(name="qkv", bufs=1) as qkvp,
            tc.tile_pool(name="qc", bufs=2) as qcp,
            tc.tile_pool(name="aout", bufs=2) as aop,
            tc.tile_pool(name="ro", bufs=2) as rop,
            tc.tile_pool(name="xs", bufs=5) as xsp,
            tc.tile_pool(name="stage", bufs=4) as stagep,
            tc.tile_pool(name="att", bufs=4) as attp,
            tc.tile_pool(name="acc", bufs=4, space="PSUM") as accp,
            tc.tile_pool(name="satt", bufs=2, space="PSUM") as sattp,
            tc.tile_pool(name="attden", bufs=2, space="PSUM") as adp,
        ):
            # ---- resident weights / tables -------------------------------
            # wo and x(0) pieces interleaved on one queue in need-time order:
            # the q-pass consumes k-chunk k at ~0.85us/chunk, so each piece
            # must land just before the PE reaches it.
            wo_s = wpool.tile([P, NKC, JPC], BF16)
            x0q = [xsp.tile([P, 8, 512], BF16, name=f"xt0_{q}", tag="xs")
                   for q in range(4)]

            def wo_piece(klo, khi):
                nc.sync.dma_start(
                    out=wo_s[:, klo:khi, :],
                    in_=woT[klo * P:khi * P, :].rearrange("(k p) j -> p k j",
                                                          p=P))

            def x0_piece(q, klo, khi):
                nc.sync.dma_start(
                    out=x0q[q][:, klo:khi, :],
                    in_=xTb[0, (8 * q + klo) * P:(8 * q + khi) * P,
                            :].rearrange("(k p) t -> p k t", p=P))

            wo_piece(0, 1)
            x0_piece(0, 0, 2)
            wo_piece(1, 2)
            x0_piece(0, 2, 8)
            wo_piece(2, 4)
            x0_piece(1, 0, 4)
            wo_piece(4, 8)
            x0_piece(1, 4, 8)
            wo_piece(8, 12)
            x0_piece(2, 0, 4)
            wo_piece(12, 16)
            x0_piece(2, 4, 8)
            wo_piece(16, 20)
            x0_piece(3, 0, 4)
            wo_piece(20, 24)
            x0_piece(3, 4, 8)
            wo_piece(24, 28)
            wk_s = wpool.tile([P, NKC, HD], BF16)
            nc.sync.dma_start(out=wk_s, in_=wkT[:, :].rearrange(
                "(k p) j -> p k j", p=P))
            wo_piece(28, 32)
            wv_s = wpool.tile([P, NKC, HD], BF16)
            nc.sync.dma_start(out=wv_s, in_=wvT[:, :].rearrange(
                "(k p) j -> p k j", p=P))

            cos_s = constp.tile([P, T], BF16)
            sin_s = constp.tile([P, T], BF16)
            nc.sync.dma_start(out=cos_s, in_=cost[:, :])
            nc.sync.dma_start(out=sin_s, in_=sint[:, :])

            rT_s = constp.tile([P, P], BF16)
            nc.sync.dma_start(out=rT_s, in_=rT[:, :])
            qb_s = constp.tile([P, HPC], F32)
            qb2_s = constp.tile([P, HPC], F32)
            kb_s = constp.tile([P, 1], F32)
            vb_s = constp.tile([P, 1], F32)
            nc.sync.dma_start(out=qb_s, in_=qb[:, :])
            nc.sync.dma_start(out=qb2_s, in_=qb2[:, :])
            nc.sync.dma_start(out=kb_s, in_=kb[:, :])
            nc.sync.dma_start(out=vb_s, in_=vb[:, :])

            # row-slice of wo for the contraction-sharded output projection
            # (loaded lazily -- per-m DMAs are emitted inside the n==0 body so
            # they don't compete with x/wo for early DMA bandwidth)
            wo3_s = wpool.tile([P, HPC, DIM], BF16)

            ones_s = constp.tile([P, P], BF16)
            nc.vector.memset(ones_s, 1.0)
            ident_s = constp.tile([P, P], BF16)
            make_identity(nc, ident_s)

            # persistent K/V in rope-d transposed layout (Q is per-chunk)
            kT_s = qkvp.tile([P, T], BF16)        # [hd, t]
            vN_s = qkvp.tile([P, TKC, HD], BF16)  # [tk%128, tk//128, hd]

            # ---- phase 1: projections for all t-chunks ------------------
            def emit_x(n):
                # x for one t-chunk: four 8-k-chunk quarter tiles
                tiles = []
                for q in range(4):
                    ks = slice(q * 8 * P, (q + 1) * 8 * P)
                    xq = xsp.tile([P, 8, 512], BF16, name=f"xt{n}_{q}",
                                  tag="xs")
                    nc.sync.dma_start(
                        out=xq,
                        in_=xTb[n, ks, :].rearrange("(k p) t -> p k t", p=P))
                    tiles.append(xq)
                return tiles

            # chunk-0 mask tiles preloaded during the projection phase: the
            # first attention pairs of chunk 0 (emitted last) need them with
            # no slack, and every k-tile of chunk 0 crosses the diagonal.
            pre_mtiles = {}

            next_xtq = None
            for n in range(NT):
                ts = slice(n * 512, (n + 1) * 512)
                xtq = next_xtq

                if n == 0:
                    xtq = x0q

                def xt_sl(k, xtq=xtq):
                    return xtq[k // 8][:, k % 8, :]

                acc_tiles = []
                for m in range(HPC + 2):  # 4 q-head tiles, k, v
                    pacc = accp.tile([P, 512], F32, name=f"pacc{n}_{m}",
                                     tag="acc")
                    acc_tiles.append(pacc)
                # q-pass
                for k in range(NKC):
                    for m in range(HPC):
                        nc.tensor.matmul(
                            acc_tiles[m],
                            lhsT=wo_s[:, k, m * P:(m + 1) * P],
                            rhs=xt_sl(k),
                            start=(k == 0),
                            stop=(k == NKC - 1),
                        )
                # q biases on Act while the kv-pass runs on PE
                braw_q = []
                for m in range(HPC):
                    braw = stagep.tile([P, 512], BF16, name=f"braw{n}_{m}",
                                       tag="braw")
                    nc.scalar.add(braw, acc_tiles[m], qb_s[:, m:m + 1])
                    braw_q.append(braw)
                # rope table slices for this chunk (small, late-need DMAs)
                nc.sync.dma_start(out=cos_s[:, ts], in_=cost[:, ts])
                nc.sync.dma_start(out=sin_s[:, ts], in_=sint[:, ts])
                # k-pass then v-pass: the k accumulator finishes at the
                # halfway point, so its bias (and the psum bank the third
                # rotation matmul reuses) is ready before the rotations
                for k in range(NKC):
                    nc.tensor.matmul(
                        acc_tiles[HPC], lhsT=wk_s[:, k, :], rhs=xt_sl(k),
                        start=(k == 0), stop=(k == NKC - 1),
                    )
                for k in range(NKC):
                    nc.tensor.matmul(
                        acc_tiles[HPC + 1], lhsT=wv_s[:, k, :], rhs=xt_sl(k),
                        start=(k == 0), stop=(k == NKC - 1),
                    )
                # issue the next chunk's x loads now (Pool queue is free of
                # collective waits during the projection phase)
                if n + 1 < NT:
                    next_xtq = emit_x(n + 1)
                if n == 0:
                    # wo row-slice for the output projection: not needed for
                    # ~150us, so loaded after the startup-critical DMAs
                    for m in range(HPC):
                        nc.sync.dma_start(out=wo3_s[:, m, :],
                                          in_=woR[m * P:(m + 1) * P, :])
                    for k in range(TKC):
                        if klass[k, 0] == MSK_ADD:
                            mt = attp.tile([P, 512], BF16, name=f"mt0_{k}",
                                           tag="mskpre", bufs=4)
                            nc.sync.dma_start(
                                out=mt, in_=maskTb[0, k * P:(k + 1) * P, :])
                            pre_mtiles[k] = mt
                # k bias first: it frees the psum bank that the third
                # q-rotation matmul reuses; v bias next for the transposes
                brawk = stagep.tile([P, 512], BF16, name=f"brawk{n}",
                                    tag="braw")
                nc.scalar.add(brawk, acc_tiles[HPC], kb_s[:, 0:1])
                v_st = stagep.tile([P, 512], BF16, name=f"vst{n}", tag="braw")
                nc.scalar.add(v_st, acc_tiles[HPC + 1], vb_s[:, 0:1])

                # rotation matmuls for q tiles + k tile (PE, after kv-pass)
                qc_s = qcp.tile([P, HPC, 512], BF16, name=f"qc{n}", tag="qc")
                rot_q = []
                for m in range(HPC):
                    rot_ps = accp.tile([P, 512], F32, name=f"rot{n}_{m}",
                                       tag="acc")
                    nc.tensor.matmul(rot_ps, lhsT=rT_s, rhs=braw_q[m],
                                     start=True, stop=True)
                    rot_q.append(rot_ps)
                rot_k = accp.tile([P, 512], F32, name=f"rotk{n}", tag="acc")
                nc.tensor.matmul(rot_k, lhsT=rT_s, rhs=brawk,
                                 start=True, stop=True)
                # v transpose into natural layout
                for j in range(4):
                    vt_ps = accp.tile([P, P], BF16, name=f"vt{n}_{j}",
                                      tag="acc")
                    nc.tensor.transpose(vt_ps, v_st[:, j * P:(j + 1) * P],
                                        ident_s)
                    nc.scalar.copy(vN_s[:, n * 4 + j, :], vt_ps)

                # rope combine on DVE (all-bf16 for 2x mode where possible)
                for m in range(HPC):
                    dst = qc_s[:, m, :]
                    tmp = stagep.tile([P, 512], BF16, name=f"tmp{n}_{m}",
                                      tag="stage")
                    nc.vector.tensor_mul(tmp, rot_q[m], sin_s[:, ts])
                    nc.vector.tensor_mul(dst, braw_q[m], cos_s[:, ts])
                    nc.vector.tensor_add(dst, dst, tmp)
                tmpk = stagep.tile([P, 512], BF16, name=f"tmpk{n}", tag="stage")
                nc.vector.tensor_mul(tmpk, rot_k, sin_s[:, ts])
                nc.vector.tensor_mul(kT_s[:, ts], brawk, cos_s[:, ts])
                nc.vector.tensor_add(kT_s[:, ts], kT_s[:, ts], tmpk)

                # ---- attention, partial out-proj, ReduceScatter ------
                c = n
                act_ks = [k for k in range(TKC) if klass[k, c] != MSK_SKIP]
                add_ks = [k for k in act_ks if klass[k, c] == MSK_ADD]
                if c == 0:
                    mtiles = pre_mtiles
                else:
                    mtiles = {}
                    for k in add_ks:
                        mt = attp.tile([P, 512], BF16, name=f"mt{c}_{k}",
                                       tag="msk", bufs=max(2, len(add_ks) + 1))
                        nc.sync.dma_start(
                            out=mt, in_=maskTb[c, k * P:(k + 1) * P, :])
                        mtiles[k] = mt

                attn_c = aop.tile([P, HPC, 512], BF16, name=f"ac{c}", tag="ac")
                nact = len(act_ks)
                pairs = [(h, i, k) for h in range(HPC)
                         for i, k in enumerate(act_ks)]
                state = {}

                jctr = [0]

                def emit_score(h, i, k, c=c, qc_s=qc_s, mtiles=mtiles,
                               state=state, jctr=jctr):
                    off = 0 if i == 0 else int(col0[k, c])
                    # every third score borrows a psum bank from the (idle
                    # during attention) projection pool: 3 scores in flight
                    # cover the exp round-trip latency
                    j = jctr[0]
                    jctr[0] += 1
                    pool, tg = ((accp, "acc") if j % 3 == 2
                                else (sattp, "satt"))
                    s_ps = pool.tile([P, 512], F32, name=f"sps{c}_{h}_{k}",
                                     tag=tg)
                    nc.tensor.matmul(
                        s_ps[:, off:],
                        lhsT=kT_s[:, k * P:(k + 1) * P],
                        rhs=qc_s[:, h, off:],
                        start=True, stop=True,
                    )
                    if k in mtiles:
                        nc.vector.tensor_add(s_ps[:, off:], s_ps[:, off:],
                                             mtiles[k][:, off:])
                    e_sb = attp.tile([P, 512], BF16, name=f"e{c}_{h}_{k}",
                                     tag="exp", bufs=6)
                    # exp(SCALE * s + mask): mask was pre-divided by
                    # SCALE on the host, so the add can happen upstream.
                    nc.scalar.activation(
                        e_sb[:, off:], s_ps[:, off:],
                        mybir.ActivationFunctionType.Exp, scale=SCALE)
                    state[(h, i)] = (e_sb, off)

                def emit_avden(h, i, c=c, act_ks=act_ks, nact=nact,
                               state=state, attn_c=attn_c):
                    if i == 0:
                        state[h, "apv"] = adp.tile(
                            [P, 512], F32, name=f"apv{c}_{h}", tag="attden")
                        state[h, "den"] = adp.tile(
                            [P, 512], F32, name=f"den{c}_{h}", tag="attden")
                    e_sb, off = state.pop((h, i))
                    k = act_ks[i]
                    nc.tensor.matmul(
                        state[h, "apv"][:, off:], lhsT=vN_s[:, k, :],
                        rhs=e_sb[:, off:],
                        start=(i == 0), stop=(i == nact - 1),
                    )
                    nc.tensor.matmul(
                        state[h, "den"][:, off:], lhsT=ones_s,
                        rhs=e_sb[:, off:],
                        start=(i == 0), stop=(i == nact - 1),
                    )
                    if i == nact - 1:
                        rcp = attp.tile([P, 512], F32, name=f"rcp{c}_{h}",
                                        tag="rcp", bufs=2)
                        nc.vector.reciprocal(rcp, state.pop((h, "den")))
                        nc.vector.tensor_mul(attn_c[:, h, :],
                                             state.pop((h, "apv")), rcp)

                # software-pipelined emission with lookahead 3: three
                # scores sit between a pair's score and its exp-dependent
                # matmuls, covering the exp latency and (at head boundaries)
                # the DVE normalize that frees the psum accumulator slots.
                LA = 3
                for j, (h, i, k) in enumerate(pairs):
                    emit_score(h, i, k)
                    if j >= LA:
                        ph, pi, _ = pairs[j - LA]
                        emit_avden(ph, pi)
                for j in range(max(0, len(pairs) - LA), len(pairs)):
                    emit_avden(*pairs[j][:2])

                # partial output projection: contraction over this core's 512
                # attention dims, all 4096 output dims; psum banks borrowed
                # from the (idle) attention pools.  Output dims are split in
                # two halves with one ReduceScatter each so the first
                # collective fires while the second half is still on the PE.
                # (8, 24) o-tile split: the small leading collective
                # clears the device before the big piece's data is ready,
                # minimizing the exposed tail after the last chunk
                rs_splits = {7: (0, 8), 31: (8, 32)}
                for jo in range(NKC):
                    po = (sattp if jo % 2 == 0 else adp).tile(
                        [P, 512], F32, name=f"po{c}_{jo}",
                        tag="satt" if jo % 2 == 0 else "attden")
                    for m in range(HPC):
                        nc.tensor.matmul(
                            po,
                            lhsT=wo3_s[:, m, jo * P:(jo + 1) * P],
                            rhs=attn_c[:, m, :],
                            start=(m == 0),
                            stop=(m == HPC - 1),
                        )
                    osb = attp.tile([P, 512], BF16, name=f"osb{c}_{jo}",
                                    tag="osb", bufs=4)
                    # GPSIMD cannot read PSUM; alternate DVE/Act for the
                    # psum->sbuf downcast copies
                    if jo % 2 == 0:
                        nc.vector.tensor_copy(osb, po)
                    else:
                        nc.scalar.copy(osb, po)
                    nc.sync.dma_start(out=rs_in[c, jo * P:(jo + 1) * P, :],
                                      in_=osb)
                    if jo in rs_splits:
                        lo, hi = rs_splits[jo]
                        nc.gpsimd.collective_compute(
                            "ReduceScatter",
                            mybir.AluOpType.add,
                            replica_groups=[list(range(NCORE))],
                            ins=[rs_in[c, lo * P:hi * P, :]],
                            outs=[rs_out[c, lo * P // NCORE:
                                          hi * P // NCORE, :]],
                        )


            # ---- phase 4: bias + writeback (SP + DVE, after everything) --
            # core cix's rows for half hf map to output dims
            # 2048*hf + 256*cix + [0, 256); bias slices (qb2) and the
            # host-side reassembly account for this.  Emitted last so the
            # rs_out reads (which wait on the collectives) never block the
            # mask/osb DMAs on the SP queue.
            # tile_wait_until pins these to the end of every engine queue in
            # the scheduler's virtual clock: a phase-4 op waiting on a
            # collective must never head-block mask/x DMAs or rope ops.
            with tc.tile_wait_until(10.0):
                for c in range(NT):
                    ro_s = rop.tile([P, HPC, 512], BF16, name=f"ro{c}",
                                    tag="ro", bufs=2)
                    # piece A: rs_out rows [0,128) = 1 tile; piece B:
                    # rows [128,512) = 3 tiles
                    nc.sync.dma_start(out=ro_s[:, 0, :],
                                      in_=rs_out[c, 0:P, :])
                    o_sb = stagep.tile([P, 512], F32, name=f"o{c}_0",
                                       tag="ostage", bufs=2)
                    nc.vector.tensor_scalar_add(o_sb, ro_s[:, 0, :],
                                                qb2_s[:, 0:1])
                    nc.sync.dma_start(out=outTb[c, 0:P, :], in_=o_sb)
                    nc.sync.dma_start(
                        out=ro_s[:, 1:4, :],
                        in_=rs_out[c, P:JPC, :].rearrange(
                            "(m p) t -> p m t", p=P))
                    for m in range(1, HPC):
                        o_sb = stagep.tile([P, 512], F32, name=f"o{c}_{m}",
                                           tag="ostage", bufs=2)
                        nc.vector.tensor_scalar_add(o_sb, ro_s[:, m, :],
                                                    qb2_s[:, m:m + 1])
                        nc.sync.dma_start(out=outTb[c, m * P:(m + 1) * P, :],
                                          in_=o_sb)

    legalize_waits(nc)
    return nc


def _marshal_inputs(x, freqs_cos, freqs_sin, mask, wk_w, wk_b, wv_w, wv_b,
                    wo_w, wo_b):
    bf = ml_dtypes.bfloat16
    x = np.asarray(x, np.float32)
    mask = np.asarray(mask, np.float32)
    cos = np.asarray(freqs_cos, np.float32)
    sin = np.asarray(freqs_sin, np.float32)
    wk_w = np.asarray(wk_w, np.float32)
    wv_w = np.asarray(wv_w, np.float32)
    wo_w = np.asarray(wo_w, np.float32)
    wk_b = np.asarray(wk_b, np.float32)
    wv_b = np.asarray(wv_b, np.float32)
    wo_b = np.asarray(wo_b, np.float32)

    xT = x.reshape(T, DIM).T                       # (DIM, T)
    xTb = np.ascontiguousarray(
        xT.reshape(DIM, NT, 512).transpose(1, 0, 2).astype(bf))
    # mask applied on-device as exp(SCALE*s + SCALE*maskT): pre-divide, and
    # reblock (tq-chunk, tk, tq') so every mask tile DMA is contiguous
    maskT = mask.T / np.float32(SCALE)             # (tk, tq)
    maskTb = np.ascontiguousarray(
        maskT.reshape(T, NT, 512).transpose(1, 0, 2).astype(bf))

    cos2 = np.repeat(cos.T, 2, axis=0)  # (128, T): rows 2i,2i+1 = cos[:, i]
    sin2 = np.repeat(sin.T, 2, axis=0)

    # rotation matmul constant: out = R @ q with rot[2i] = -q[2i+1],
    # rot[2i+1] = q[2i]; lhsT layout (R transposed).
    RT = np.zeros((P, P), np.float32)
    idx = np.arange(0, P, 2)
    RT[idx + 1, idx] = -1.0
    RT[idx, idx + 1] = 1.0

    common = dict(
        xTb=xTb, maskTb=maskTb,
        cost=np.ascontiguousarray(cos2.astype(bf)),
        sint=np.ascontiguousarray(sin2.astype(bf)),
        rT=RT.astype(bf),
    )

    woT_full = wo_w.T  # (DIM in, DIM out): woT_full[d, o] = wo_w[o, d]
    in_maps = []
    for cix in range(NCORE):
        jlo = cix * JPC
        klo = cix * HD
        m = dict(common)
        m["woT"] = np.ascontiguousarray(wo_w[jlo:jlo + JPC, :].T.astype(bf))
        m["woR"] = np.ascontiguousarray(woT_full[jlo:jlo + JPC, :].astype(bf))
        m["wkT"] = np.ascontiguousarray(wk_w[klo:klo + HD, :].T.astype(bf))
        m["wvT"] = np.ascontiguousarray(wv_w[klo:klo + HD, :].T.astype(bf))
        m["qb"] = np.ascontiguousarray(wo_b[jlo:jlo + JPC].reshape(HPC, P).T)
        # phase-4 bias for the (8, 24)-o-tile ReduceScatter split:
        # row block 0 holds output dims 128*cix + [0, 128); row block
        # m in {1,2,3} holds 1024 + 384*cix + 128*(m-1) + [0, 128)
        qb2 = np.empty((P, HPC), np.float32)
        qb2[:, 0] = wo_b[128 * cix:128 * cix + P]
        for m2 in range(3):
            base = 1024 + 384 * cix + 128 * m2
            qb2[:, 1 + m2] = wo_b[base:base + P]
        m["qb2"] = np.ascontiguousarray(qb2)
        m["kb"] = np.ascontiguousarray(wk_b[klo:klo + HD].reshape(1, P).T)
        m["vb"] = np.ascontiguousarray(wv_b[klo:klo + HD].reshape(1, P).T)
        in_maps.append(m)
    return in_maps, mask


def run(inputs, trace=False):
    """Build, run on 8 cores, return (full_output, BassKernelResults)."""
    in_maps, mask = _marshal_inputs(
        inputs["x"], inputs["freqs_cos"], inputs["freqs_sin"], inputs["mask"],
        inputs["wk_w"], inputs["wk_b"], inputs["wv_w"], inputs["wv_b"],
        inputs["wo_w"], inputs["wo_b"])
    klass, col0 = _classify_mask(mask)
    nc = _build_module(klass, col0)
    res = run_bass_kernel_spmd(nc, in_maps, core_ids=list(range(NCORE)),
                               trace=trace)
    out = np.empty((DIM, T), np.float32)
    for cix in range(NCORE):
        ob = res.results[cix]["outTb"]          # (NT, JPC, 512)
        for n in range(NT):
            cols = slice(n * 512, (n + 1) * 512)
            out[128 * cix:128 * cix + P, cols] = ob[n, 0:P]
            for m2 in range(3):
                base = 1024 + 384 * cix + 128 * m2
                out[base:base + P, cols] = ob[n, (1 + m2) * P:(2 + m2) * P]
    out = out.T  # (T, DIM)
    return np.ascontiguousarray(out[None, :, :]).astype(np.float32), res


def kernel(**inputs):
    out, _ = run(inputs, trace=False)
    return out


# revision 24
# speedup vs baseline: 1.0301x; 1.0301x over previous
"""Tensor-parallel GQA attention prefill (B=1, T=2048, D=4096, 32 q-heads /
8 kv-heads) for 8 Trainium2 NeuronCores.

Sharding: head-parallel.  Core c owns q-heads [4c, 4c+4) and kv-head c.
  phase 1: Q/K/V projections in transposed layout (head-dim on partitions),
           RoPE applied via a rotation-matmul + two table multiplies.
  phase 2: per-head attention with scores held transposed (tk on
           partitions); softmax denominators come from a ones-matmul;
           fully-masked tiles are skipped (host inspects the mask tensor).
  phase 3: output projection sharded over the CONTRACTION dim: each core
           multiplies its local attention slice (512 rows) against its
           512-row slice of wo, producing partial sums for ALL 4096 output
           dims; a per-tq-chunk ReduceScatter (bf16) then both sums the
           partials and hands each core its own 512 output rows.
  phase 4: bias add + writeback of the scattered result.

Matmul operands are bf16 (fp32 accumulation in PSUM); measured end-to-end
error vs the fp32 reference is ~4e-3 relative.

NOTE: faithful to the reference "bug" -- the q projection uses wo_w/wo_b.
"""

import numpy as np
import ml_dtypes

import bass_rust
import concourse.bass as bass
import concourse.mybir as mybir
import concourse.tile as tile
from concourse.bass_utils import run_bass_kernel_spmd
from concourse.masks import make_identity

# problem constants (self-contained; do not read spec.json)
DIM = 4096
NH = 32
NKV = 8
HD = 128
T = 2048
NCORE = 8
HPC = NH // NCORE      # 4 q heads per core
JPC = HPC * HD         # 512 output columns per core
P = 128
NT = T // 512          # 4 free-dim chunks of 512
NKC = DIM // P         # 32 contraction chunks in the projections
TKC = T // P           # 16 tk chunks in attention
SCALE = 1.0 / float(np.sqrt(HD))

F32 = mybir.dt.float32
BF16 = mybir.dt.bfloat16

# mask tile classification
MSK_SKIP, MSK_ZERO, MSK_ADD = 0, 1, 2


def legalize_waits(nc, max_waits=1):
    """Hoist excess on_wait conditions onto preceding nop instructions.

    This walrus build rejects instructions carrying more than a couple of
    sync-wait commands; engines execute their queue in order, so a nop that
    waits immediately before the real instruction is equivalent.
    """
    n_new = 0
    for f in nc.m.functions:
        for bb in f.blocks:
            insts = bb.instructions
            new = []
            for ins in list(insts):
                si = ins.sync_info
                waits = list(si.on_wait) if si is not None and si.on_wait else []
                if len(waits) > max_waits:
                    hoist = waits[:-max_waits]
                    keep = waits[-max_waits:]
                    for j in range(0, len(hoist), max_waits):
                        chunk = hoist[j:j + max_waits]
                        nop = mybir.InstNoOp(
                            name=f"{ins.name}_hw{j}",
                            engine=ins.engine,
                            sync_info=bass_rust.SyncInfo(
                                on_wait=chunk, on_update=[]),
                        )
                        new.append(nop)
                        n_new += 1
                    ins.sync_info = bass_rust.SyncInfo(
                        on_wait=keep,
                        on_update=list(si.on_update) if si.on_update else [])
                new.append(ins)
            insts.clear()
            insts.extend(new)
    return n_new


def _classify_mask(mask):
    """Per (tk-chunk, tq-chunk-of-512) classification of the additive mask.

    Returns (klass, col0) where col0[k, c] is the first tq column (multiple
    of 128) of the chunk that is not fully masked -- matmuls/exp for the
    columns before it are skipped (their softmax weights are exactly 0).
    """
    klass = np.empty((TKC, NT), dtype=np.int32)
    col0 = np.zeros((TKC, NT), dtype=np.int32)
    for k in range(TKC):
        for c in range(NT):
            blk = mask[c * 512:(c + 1) * 512, k * P:(k + 1) * P]
            mx = float(blk.max())
            mn = float(blk.min())
            if mx < -80.0:
                klass[k, c] = MSK_SKIP
                continue
            if mx == 0.0 and mn == 0.0:
                klass[k, c] = MSK_ZERO
            else:
                klass[k, c] = MSK_ADD
            # leading fully-masked tq columns, rounded down to 128
            colmax = blk.max(axis=1)          # per-tq-row max over this tile
            nz = np.nonzero(colmax >= -80.0)[0]
            first = int(nz[0]) if len(nz) else 0
            first = (first // P) * P
            # only safe to skip if every column before `first` is fully masked
            if first > 0 and float(blk[:first].max()) < -80.0:
                col0[k, c] = first
    # never allow a fully-empty (all-skip) tq chunk; keep one tile live
    for c in range(NT):
        if all(klass[k, c] == MSK_SKIP for k in range(TKC)):
            klass[min(c * 4, TKC - 1), c] = MSK_ADD
    return klass, col0


def _build_module(klass, col0, phases=(1, 2, 25, 3)):
    nc = bass.Bass()

    # inputs are pre-reblocked on the host so every DMA is contiguous
    xTb = nc.declare_dram_parameter("xTb", [NT, DIM, 512], BF16, isOutput=False)
    woT = nc.declare_dram_parameter("woT", [DIM, JPC], BF16, isOutput=False)
    woR = nc.declare_dram_parameter("woR", [JPC, DIM], BF16, isOutput=False)
    wkT = nc.declare_dram_parameter("wkT", [P, NKC * HD], BF16,
                                    isOutput=False)
    wvT = nc.declare_dram_parameter("wvT", [P, NKC * HD], BF16,
                                    isOutput=False)
    qb = nc.declare_dram_parameter("qb", [P, HPC], F32, isOutput=False)
    qb2 = nc.declare_dram_parameter("qb2", [P, HPC], F32, isOutput=False)
    kb = nc.declare_dram_parameter("kb", [P, 1], F32, isOutput=False)
    vb = nc.declare_dram_parameter("vb", [P, 1], F32, isOutput=False)
    maskTb = nc.declare_dram_parameter("maskTb", [NT, T, 512], BF16,
                                       isOutput=False)
    cost = nc.declare_dram_parameter("cost", [P, T], BF16, isOutput=False)
    sint = nc.declare_dram_parameter("sint", [P, T], BF16, isOutput=False)
    rT = nc.declare_dram_parameter("rT", [P, P], BF16, isOutput=False)
    outTb = nc.declare_dram_parameter("outTb", [NT, JPC, 512], F32,
                                      isOutput=True)

    rs_in = nc.dram_tensor("rs_in", [NT, NCORE * JPC, 512], BF16)
    rs_out = nc.dram_tensor("rs_out", [NT, JPC, 512], BF16)

    with tile.TileContext(nc) as tc:
        with (
            tc.tile_pool(name="wpool", bufs=1) as wpool,
            tc.tile_pool(name="const", bufs=1) as constp,
            tc.tile_pool(name="qkv", bufs=1) as qkvp,
            tc.tile_pool(name="qc", bufs=2) as qcp,
            tc.tile_pool(name="aout", bufs=2) as aop,
            tc.tile_pool(name="ro", bufs=2) as rop,
            tc.tile_pool(name="xs", bufs=5) as xsp,
            tc.tile_pool(name="stage", bufs=4) as stagep,
            tc.tile_pool(name="att", bufs=4) as attp,
            tc.tile_pool(name="acc", bufs=4, space="PSUM") as accp,
            tc.tile_pool(name="satt", bufs=2, space="PSUM") as sattp,
            tc.tile_pool(name="attden", bufs=2, space="PSUM") as adp,
        ):
            # ---- resident weights / tables -------------------------------
            # wo and x(0) pieces interleaved on one queue in need-time order:
            # the q-pass consumes k-chunk k at ~0.85us/chunk, so each piece
            # must land just before the PE reaches it.
            wo_s = wpool.tile([P, NKC, JPC], BF16)
            x0q = [xsp.tile([P, 8, 512], BF16, name=f"xt0_{q}", tag="xs")
                   for q in range(4)]

            def wo_piece(klo, khi):
                nc.sync.dma_start(
                    out=wo_s[:, klo:khi, :],
                    in_=woT[klo * P:khi * P, :].rearrange("(k p) j -> p k j",
                                                          p=P))

            def x0_piece(q, klo, khi):
                nc.sync.dma_start(
                    out=x0q[q][:, klo:khi, :],
                    in_=xTb[0, (8 * q + klo) * P:(8 * q + khi) * P,
                            :].rearrange("(k p) t -> p k t", p=P))

            wo_piece(0, 1)
            x0_piece(0, 0, 2)
            wo_piece(1, 2)
            x0_piece(0, 2, 8)
            wo_piece(2, 4)
            x0_piece(1, 0, 4)
            wo_piece(4, 8)
            x0_piece(1, 4, 8)
            wo_piece(8, 12)
            x0_piece(2, 0, 4)
            wo_piece(12, 16)
            x0_piece(2, 4, 8)
            wo_piece(16, 20)
            x0_piece(3, 0, 4)
            wo_piece(20, 24)
            x0_piece(3, 4, 8)
            wk_s = wpool.tile([P, NKC, HD], BF16)
            nc.sync.dma_start(out=wk_s, in_=wkT[:, :])
            wo_piece(24, 28)
            wo_piece(28, 32)
            wv_s = wpool.tile([P, NKC, HD], BF16)
            nc.sync.dma_start(out=wv_s, in_=wvT[:, :])

            cos_s = constp.tile([P, T], BF16)
            sin_s = constp.tile([P, T], BF16)
            nc.sync.dma_start(out=cos_s, in_=cost[:, :])
            nc.sync.dma_start(out=sin_s, in_=sint[:, :])

            rT_s = constp.tile([P, P], BF16)
            nc.sync.dma_start(out=rT_s, in_=rT[:, :])
            qb_s = constp.tile([P, HPC], F32)
            qb2_s = constp.tile([P, HPC], F32)
            kb_s = constp.tile([P, 1], F32)
            vb_s = constp.tile([P, 1], F32)
            nc.sync.dma_start(out=qb_s, in_=qb[:, :])
            nc.sync.dma_start(out=qb2_s, in_=qb2[:, :])
            nc.sync.dma_start(out=kb_s, in_=kb[:, :])
            nc.sync.dma_start(out=vb_s, in_=vb[:, :])

            # row-slice of wo for the contraction-sharded output projection
            # (loaded lazily -- per-m DMAs are emitted inside the n==0 body so
            # they don't compete with x/wo for early DMA bandwidth)
            wo3_s = wpool.tile([P, HPC, DIM], BF16)

            ones_s = constp.tile([P, P], BF16)
            nc.vector.memset(ones_s, 1.0)
            ident_s = constp.tile([P, P], BF16)
            make_identity(nc, ident_s)

            # persistent K/V in rope-d transposed layout (Q is per-chunk)
            kT_s = qkvp.tile([P, T], BF16)        # [hd, t]
            vN_s = qkvp.tile([P, TKC, HD], BF16)  # [tk%128, tk//128, hd]

            # ---- phase 1: projections for all t-chunks ------------------
            def emit_x(n):
                # x for one t-chunk: four 8-k-chunk quarter tiles
                tiles = []
                for q in range(4):
                    ks = slice(q * 8 * P, (q + 1) * 8 * P)
                    xq = xsp.tile([P, 8, 512], BF16, name=f"xt{n}_{q}",
                                  tag="xs")
                    nc.sync.dma_start(
                        out=xq,
                        in_=xTb[n, ks, :].rearrange("(k p) t -> p k t", p=P))
                    tiles.append(xq)
                return tiles

            # chunk-0 mask tiles preloaded during the projection phase: the
            # first attention pairs of chunk 0 (emitted last) need them with
            # no slack, and every k-tile of chunk 0 crosses the diagonal.
            pre_mtiles = {}

            next_xtq = None
            for n in range(NT):
                ts = slice(n * 512, (n + 1) * 512)
                xtq = next_xtq

                if n == 0:
                    xtq = x0q

                def xt_sl(k, xtq=xtq):
                    return xtq[k // 8][:, k % 8, :]

                acc_tiles = []
                for m in range(HPC + 2):  # 4 q-head tiles, k, v
                    pacc = accp.tile([P, 512], F32, name=f"pacc{n}_{m}",
                                     tag="acc")
                    acc_tiles.append(pacc)
                # q-pass
                for k in range(NKC):
                    for m in range(HPC):
                        nc.tensor.matmul(
                            acc_tiles[m],
                            lhsT=wo_s[:, k, m * P:(m + 1) * P],
                            rhs=xt_sl(k),
                            start=(k == 0),
                            stop=(k == NKC - 1),
                        )
                # q biases on Act while the kv-pass runs on PE
                braw_q = []
                for m in range(HPC):
                    braw = stagep.tile([P, 512], BF16, name=f"braw{n}_{m}",
                                       tag="braw")
                    nc.scalar.add(braw, acc_tiles[m], qb_s[:, m:m + 1])
                    braw_q.append(braw)
                # rope table slices for this chunk (small, late-need DMAs)
                nc.sync.dma_start(out=cos_s[:, ts], in_=cost[:, ts])
                nc.sync.dma_start(out=sin_s[:, ts], in_=sint[:, ts])
                # k-pass then v-pass: the k accumulator finishes at the
                # halfway point, so its bias (and the psum bank the third
                # rotation matmul reuses) is ready before the rotations
                for k in range(NKC):
                    nc.tensor.matmul(
                        acc_tiles[HPC], lhsT=wk_s[:, k, :], rhs=xt_sl(k),
                        start=(k == 0), stop=(k == NKC - 1),
                    )
                for k in range(NKC):
                    nc.tensor.matmul(
                        acc_tiles[HPC + 1], lhsT=wv_s[:, k, :], rhs=xt_sl(k),
                        start=(k == 0), stop=(k == NKC - 1),
                    )
                # issue the next chunk's x loads now (Pool queue is free of
                # collective waits during the projection phase)
                if n + 1 < NT:
                    next_xtq = emit_x(n + 1)
                if n == 0:
                    # wo row-slice for the output projection: not needed for
                    # ~150us, so loaded after the startup-critical DMAs
                    for m in range(HPC):
                        nc.sync.dma_start(out=wo3_s[:, m, :],
                                          in_=woR[m * P:(m + 1) * P, :])
                    for k in range(TKC):
                        if klass[k, 0] == MSK_ADD:
                            mt = attp.tile([P, 512], BF16, name=f"mt0_{k}",
                                           tag="mskpre", bufs=4)
                            nc.sync.dma_start(
                                out=mt, in_=maskTb[0, k * P:(k + 1) * P, :])
                            pre_mtiles[k] = mt
                # k bias first: it frees the psum bank that the third
                # q-rotation matmul reuses; v bias next for the transposes
                brawk = stagep.tile([P, 512], BF16, name=f"brawk{n}",
                                    tag="braw")
                nc.scalar.add(brawk, acc_tiles[HPC], kb_s[:, 0:1])
                v_st = stagep.tile([P, 512], BF16, name=f"vst{n}", tag="braw")
                nc.scalar.add(v_st, acc_tiles[HPC + 1], vb_s[:, 0:1])

                # rotation matmuls for q tiles + k tile (PE, after kv-pass)
                qc_s = qcp.tile([P, HPC, 512], BF16, name=f"qc{n}", tag="qc")
                rot_q = []
                for m in range(HPC):
                    rot_ps = accp.tile([P, 512], F32, name=f"rot{n}_{m}",
                                       tag="acc")
                    nc.tensor.matmul(rot_ps, lhsT=rT_s, rhs=braw_q[m],
                                     start=True, stop=True)
                    rot_q.append(rot_ps)
                rot_k = accp.tile([P, 512], F32, name=f"rotk{n}", tag="acc")
                nc.tensor.matmul(rot_k, lhsT=rT_s, rhs=brawk,
                                 start=True, stop=True)
                # v transpose into natural layout
                for j in range(4):
                    vt_ps = accp.tile([P, P], BF16, name=f"vt{n}_{j}",
                                      tag="acc")
                    nc.tensor.transpose(vt_ps, v_st[:, j * P:(j + 1) * P],
                                        ident_s)
                    nc.scalar.copy(vN_s[:, n * 4 + j, :], vt_ps)

                # rope combine on DVE (all-bf16 for 2x mode where possible)
                for m in range(HPC):
                    dst = qc_s[:, m, :]
                    tmp = stagep.tile([P, 512], BF16, name=f"tmp{n}_{m}",
                                      tag="stage")
                    nc.vector.tensor_mul(tmp, rot_q[m], sin_s[:, ts])
                    nc.vector.tensor_mul(dst, braw_q[m], cos_s[:, ts])
                    nc.vector.tensor_add(dst, dst, tmp)
                tmpk = stagep.tile([P, 512], BF16, name=f"tmpk{n}", tag="stage")
                nc.vector.tensor_mul(tmpk, rot_k, sin_s[:, ts])
                nc.vector.tensor_mul(kT_s[:, ts], brawk, cos_s[:, ts])
                nc.vector.tensor_add(kT_s[:, ts], kT_s[:, ts], tmpk)

                # ---- attention, partial out-proj, ReduceScatter ------
                c = n
                act_ks = [k for k in range(TKC) if klass[k, c] != MSK_SKIP]
                add_ks = [k for k in act_ks if klass[k, c] == MSK_ADD]
                if c == 0:
                    mtiles = pre_mtiles
                else:
                    mtiles = {}
                    for k in add_ks:
                        mt = attp.tile([P, 512], BF16, name=f"mt{c}_{k}",
                                       tag="msk", bufs=max(2, len(add_ks) + 1))
                        nc.sync.dma_start(
                            out=mt, in_=maskTb[c, k * P:(k + 1) * P, :])
                        mtiles[k] = mt

                attn_c = aop.tile([P, HPC, 512], BF16, name=f"ac{c}", tag="ac")
                nact = len(act_ks)
                pairs = [(h, i, k) for h in range(HPC)
                         for i, k in enumerate(act_ks)]
                state = {}

                jctr = [0]

                def emit_score(h, i, k, c=c, qc_s=qc_s, mtiles=mtiles,
                               state=state, jctr=jctr):
                    off = 0 if i == 0 else int(col0[k, c])
                    # every third score borrows a psum bank from the (idle
                    # during attention) projection pool: 3 scores in flight
                    # cover the exp round-trip latency
                    j = jctr[0]
                    jctr[0] += 1
                    pool, tg = ((accp, "acc") if j % 3 == 2
                                else (sattp, "satt"))
                    s_ps = pool.tile([P, 512], F32, name=f"sps{c}_{h}_{k}",
                                     tag=tg)
                    nc.tensor.matmul(
                        s_ps[:, off:],
                        lhsT=kT_s[:, k * P:(k + 1) * P],
                        rhs=qc_s[:, h, off:],
                        start=True, stop=True,
                    )
                    if k in mtiles:
                        nc.vector.tensor_add(s_ps[:, off:], s_ps[:, off:],
                                             mtiles[k][:, off:])
                    e_sb = attp.tile([P, 512], BF16, name=f"e{c}_{h}_{k}",
                                     tag="exp", bufs=6)
                    # exp(SCALE * s + mask): mask was pre-divided by
                    # SCALE on the host, so the add can happen upstream.
                    nc.scalar.activation(
                        e_sb[:, off:], s_ps[:, off:],
                        mybir.ActivationFunctionType.Exp, scale=SCALE)
                    state[(h, i)] = (e_sb, off)

                def emit_avden(h, i, c=c, act_ks=act_ks, nact=nact,
                               state=state, attn_c=attn_c):
                    if i == 0:
                        state[h, "apv"] = adp.tile(
                            [P, 512], F32, name=f"apv{c}_{h}", tag="attden")
                        state[h, "den"] = adp.tile(
                            [P, 512], F32, name=f"den{c}_{h}", tag="attden")
                    e_sb, off = state.pop((h, i))
                    k = act_ks[i]
                    nc.tensor.matmul(
                        state[h, "apv"][:, off:], lhsT=vN_s[:, k, :],
                        rhs=e_sb[:, off:],
                        start=(i == 0), stop=(i == nact - 1),
                    )
                    nc.tensor.matmul(
                        state[h, "den"][:, off:], lhsT=ones_s,
                        rhs=e_sb[:, off:],
                        start=(i == 0), stop=(i == nact - 1),
                    )
                    if i == nact - 1:
                        rcp = attp.tile([P, 512], F32, name=f"rcp{c}_{h}",
                                        tag="rcp", bufs=2)
                        nc.vector.reciprocal(rcp, state.pop((h, "den")))
                        nc.vector.tensor_mul(attn_c[:, h, :],
                                             state.pop((h, "apv")), rcp)

                # software-pipelined emission with lookahead 3: three
                # scores sit between a pair's score and its exp-dependent
                # matmuls, covering the exp latency and (at head boundaries)
                # the DVE normalize that frees the psum accumulator slots.
                LA = 3
                for j, (h, i, k) in enumerate(pairs):
                    emit_score(h, i, k)
                    if j >= LA:
                        ph, pi, _ = pairs[j - LA]
                        emit_avden(ph, pi)
                for j in range(max(0, len(pairs) - LA), len(pairs)):
                    emit_avden(*pairs[j][:2])

                # partial output projection: contraction over this core's 512
                # attention dims, all 4096 output dims; psum banks borrowed
                # from the (idle) attention pools.  Output dims are split in
                # two halves with one ReduceScatter each so the first
                # collective fires while the second half is still on the PE.
                # (8, 24) o-tile split: the small leading collective
                # clears the device before the big piece's data is ready,
                # minimizing the exposed tail after the last chunk
                rs_splits = {7: (0, 8), 31: (8, 32)}
                for jo in range(NKC):
                    po = (sattp if jo % 2 == 0 else adp).tile(
                        [P, 512], F32, name=f"po{c}_{jo}",
                        tag="satt" if jo % 2 == 0 else "attden")
                    for m in range(HPC):
                        nc.tensor.matmul(
                            po,
                            lhsT=wo3_s[:, m, jo * P:(jo + 1) * P],
                            rhs=attn_c[:, m, :],
                            start=(m == 0),
                            stop=(m == HPC - 1),
                        )
                    osb = attp.tile([P, 512], BF16, name=f"osb{c}_{jo}",
                                    tag="osb", bufs=4)
                    # GPSIMD cannot read PSUM; alternate DVE/Act for the
                    # psum->sbuf downcast copies
                    if jo % 2 == 0:
                        nc.vector.tensor_copy(osb, po)
                    else:
                        nc.scalar.copy(osb, po)
                    nc.sync.dma_start(out=rs_in[c, jo * P:(jo + 1) * P, :],
                                      in_=osb)
                    if jo in rs_splits:
                        lo, hi = rs_splits[jo]
                        nc.gpsimd.collective_compute(
                            "ReduceScatter",
                            mybir.AluOpType.add,
                            replica_groups=[list(range(NCORE))],
                            ins=[rs_in[c, lo * P:hi * P, :]],
                            outs=[rs_out[c, lo * P // NCORE:
                                          hi * P // NCORE, :]],
                        )


            # ---- phase 4: bias + writeback (SP + DVE, after everything) --
            # core cix's rows for half hf map to output dims
            # 2048*hf + 256*cix + [0, 256); bias slices (qb2) and the
            # host-side reassembly account for this.  Emitted last so the
            # rs_out reads (which wait on the collectives) never block the
            # mask/osb DMAs on the SP queue.
            # tile_wait_until pins these to the end of every engine queue in
            # the scheduler's virtual clock: a phase-4 op waiting on a
            # collective must never head-block mask/x DMAs or rope ops.
            with tc.tile_wait_until(10.0):
                for c in range(NT):
                    ro_s = rop.tile([P, HPC, 512], BF16, name=f"ro{c}",
                                    tag="ro", bufs=2)
                    # piece A: rs_out rows [0,128) = 1 tile; piece B:
                    # rows [128,512) = 3 tiles
                    nc.sync.dma_start(out=ro_s[:, 0, :],
                                      in_=rs_out[c, 0:P, :])
                    o_sb = stagep.tile([P, 512], F32, name=f"o{c}_0",
                                       tag="ostage", bufs=2)
                    nc.vector.tensor_scalar_add(o_sb, ro_s[:, 0, :],
                                                qb2_s[:, 0:1])
                    nc.sync.dma_start(out=outTb[c, 0:P, :], in_=o_sb)
                    nc.sync.dma_start(
                        out=ro_s[:, 1:4, :],
                        in_=rs_out[c, P:JPC, :].rearrange(
                            "(m p) t -> p m t", p=P))
                    for m in range(1, HPC):
                        o_sb = stagep.tile([P, 512], F32, name=f"o{c}_{m}",
                                           tag="ostage", bufs=2)
                        nc.vector.tensor_scalar_add(o_sb, ro_s[:, m, :],
                                                    qb2_s[:, m:m + 1])
                        nc.sync.dma_start(out=outTb[c, m * P:(m + 1) * P, :],
                                          in_=o_sb)

    legalize_waits(nc)
    return nc


def _marshal_inputs(x, freqs_cos, freqs_sin, mask, wk_w, wk_b, wv_w, wv_b,
                    wo_w, wo_b):
    bf = ml_dtypes.bfloat16
    x = np.asarray(x, np.float32)
    mask = np.asarray(mask, np.float32)
    cos = np.asarray(freqs_cos, np.float32)
    sin = np.asarray(freqs_sin, np.float32)
    wk_w = np.asarray(wk_w, np.float32)
    wv_w = np.asarray(wv_w, np.float32)
    wo_w = np.asarray(wo_w, np.float32)
    wk_b = np.asarray(wk_b, np.float32)
    wv_b = np.asarray(wv_b, np.float32)
    wo_b = np.asarray(wo_b, np.float32)

    xT = x.reshape(T, DIM).T                       # (DIM, T)
    xTb = np.ascontiguousarray(
        xT.reshape(DIM, NT, 512).transpose(1, 0, 2).astype(bf))
    # mask applied on-device as exp(SCALE*s + SCALE*maskT): pre-divide, and
    # reblock (tq-chunk, tk, tq') so every mask tile DMA is contiguous
    maskT = mask.T / np.float32(SCALE)             # (tk, tq)
    maskTb = np.ascontiguousarray(
        maskT.reshape(T, NT, 512).transpose(1, 0, 2).astype(bf))

    cos2 = np.repeat(cos.T, 2, axis=0)  # (128, T): rows 2i,2i+1 = cos[:, i]
    sin2 = np.repeat(sin.T, 2, axis=0)

    # rotation matmul constant: out = R @ q with rot[2i] = -q[2i+1],
    # rot[2i+1] = q[2i]; lhsT layout (R transposed).
    RT = np.zeros((P, P), np.float32)
    idx = np.arange(0, P, 2)
    RT[idx + 1, idx] = -1.0
    RT[idx, idx + 1] = 1.0

    common = dict(
        xTb=xTb, maskTb=maskTb,
        cost=np.ascontiguousarray(cos2.astype(bf)),
        sint=np.ascontiguousarray(sin2.astype(bf)),
        rT=RT.astype(bf),
    )

    woT_full = wo_w.T  # (DIM in, DIM out): woT_full[d, o] = wo_w[o, d]
    in_maps = []
    for cix in range(NCORE):
        jlo = cix * JPC
        klo = cix * HD
        m = dict(common)
        m["woT"] = np.ascontiguousarray(wo_w[jlo:jlo + JPC, :].T.astype(bf))
        m["woR"] = np.ascontiguousarray(woT_full[jlo:jlo + JPC, :].astype(bf))
        # pre-block [DIM, HD] -> [P, NKC*HD]: partition row p holds the
        # k-chunk-major weights so one 8KB-contiguous DMA suffices
        wkTf = wk_w[klo:klo + HD, :].T.astype(bf)     # (DIM, HD)
        wvTf = wv_w[klo:klo + HD, :].T.astype(bf)
        m["wkT"] = np.ascontiguousarray(
            wkTf.reshape(NKC, P, HD).transpose(1, 0, 2).reshape(P, NKC * HD))
        m["wvT"] = np.ascontiguousarray(
            wvTf.reshape(NKC, P, HD).transpose(1, 0, 2).reshape(P, NKC * HD))
        m["qb"] = np.ascontiguousarray(wo_b[jlo:jlo + JPC].reshape(HPC, P).T)
        # phase-4 bias for the (8, 24)-o-tile ReduceScatter split:
        # row block 0 holds output dims 128*cix + [0, 128); row block
        # m in {1,2,3} holds 1024 + 384*cix + 128*(m-1) + [0, 128)
        qb2 = np.empty((P, HPC), np.float32)
        qb2[:, 0] = wo_b[128 * cix:128 * cix + P]
        for m2 in range(3):
            base = 1024 + 384 * cix + 128 * m2
            qb2[:, 1 + m2] = wo_b[base:base + P]
        m["qb2"] = np.ascontiguousarray(qb2)
        m["kb"] = np.ascontiguousarray(wk_b[klo:klo + HD].reshape(1, P).T)
        m["vb"] = np.ascontiguousarray(wv_b[klo:klo + HD].reshape(1, P).T)
        in_maps.append(m)
    return in_maps, mask


def run(inputs, trace=False):
    """Build, run on 8 cores, return (full_output, BassKernelResults)."""
    in_maps, mask = _marshal_inputs(
        inputs["x"], inputs["freqs_cos"], inputs["freqs_sin"], inputs["mask"],
        inputs["wk_w"], inputs["wk_b"], inputs["wv_w"], inputs["wv_b"],
        inputs["wo_w"], inputs["wo_b"])
    klass, col0 = _classify_mask(mask)
    nc = _build_module(klass, col0)
    res = run_bass_kernel_spmd(nc, in_maps, core_ids=list(range(NCORE)),
                               trace=trace)
    out = np.empty((DIM, T), np.float32)
    for cix in range(NCORE):
        ob = res.results[cix]["outTb"]          # (NT, JPC, 512)
        for n in range(NT):
            cols = slice(n * 512, (n + 1) * 512)
            out[128 * cix:128 * cix + P, cols] = ob[n, 0:P]
            for m2 in range(3):
                base = 1024 + 384 * cix + 128 * m2
                out[base:base + P, cols] = ob[n, (1 + m2) * P:(2 + m2) * P]
    out = out.T  # (T, DIM)
    return np.ascontiguousarray(out[None, :, :]).astype(np.float32), res


def kernel(**inputs):
    out, _ = run(inputs, trace=False)
    return out


# revision 25
# speedup vs baseline: 1.0426x; 1.0121x over previous
"""Tensor-parallel GQA attention prefill (B=1, T=2048, D=4096, 32 q-heads /
8 kv-heads) for 8 Trainium2 NeuronCores.

Sharding: head-parallel.  Core c owns q-heads [4c, 4c+4) and kv-head c.
  phase 1: Q/K/V projections in transposed layout (head-dim on partitions),
           RoPE applied via a rotation-matmul + two table multiplies.
  phase 2: per-head attention with scores held transposed (tk on
           partitions); softmax denominators come from a ones-matmul;
           fully-masked tiles are skipped (host inspects the mask tensor).
  phase 3: output projection sharded over the CONTRACTION dim: each core
           multiplies its local attention slice (512 rows) against its
           512-row slice of wo, producing partial sums for ALL 4096 output
           dims; a per-tq-chunk ReduceScatter (bf16) then both sums the
           partials and hands each core its own 512 output rows.
  phase 4: bias add + writeback of the scattered result.

Matmul operands are bf16 (fp32 accumulation in PSUM); measured end-to-end
error vs the fp32 reference is ~4e-3 relative.

NOTE: faithful to the reference "bug" -- the q projection uses wo_w/wo_b.
"""

import numpy as np
import ml_dtypes

import bass_rust
import concourse.bass as bass
import concourse.mybir as mybir
import concourse.tile as tile
from concourse.bass_utils import run_bass_kernel_spmd
from concourse.masks import make_identity

# problem constants (self-contained; do not read spec.json)
DIM = 4096
NH = 32
NKV = 8
HD = 128
T = 2048
NCORE = 8
HPC = NH // NCORE      # 4 q heads per core
JPC = HPC * HD         # 512 output columns per core
P = 128
NT = T // 512          # 4 free-dim chunks of 512
NKC = DIM // P         # 32 contraction chunks in the projections
TKC = T // P           # 16 tk chunks in attention
SCALE = 1.0 / float(np.sqrt(HD))

F32 = mybir.dt.float32
BF16 = mybir.dt.bfloat16

# mask tile classification
MSK_SKIP, MSK_ZERO, MSK_ADD = 0, 1, 2


def legalize_waits(nc, max_waits=1):
    """Hoist excess on_wait conditions onto preceding nop instructions.

    This walrus build rejects instructions carrying more than a couple of
    sync-wait commands; engines execute their queue in order, so a nop that
    waits immediately before the real instruction is equivalent.
    """
    n_new = 0
    for f in nc.m.functions:
        for bb in f.blocks:
            insts = bb.instructions
            new = []
            for ins in list(insts):
                si = ins.sync_info
                waits = list(si.on_wait) if si is not None and si.on_wait else []
                if len(waits) > max_waits:
                    hoist = waits[:-max_waits]
                    keep = waits[-max_waits:]
                    for j in range(0, len(hoist), max_waits):
                        chunk = hoist[j:j + max_waits]
                        nop = mybir.InstNoOp(
                            name=f"{ins.name}_hw{j}",
                            engine=ins.engine,
                            sync_info=bass_rust.SyncInfo(
                                on_wait=chunk, on_update=[]),
                        )
                        new.append(nop)
                        n_new += 1
                    ins.sync_info = bass_rust.SyncInfo(
                        on_wait=keep,
                        on_update=list(si.on_update) if si.on_update else [])
                new.append(ins)
            insts.clear()
            insts.extend(new)
    return n_new


def _classify_mask(mask):
    """Per (tk-chunk, tq-chunk-of-512) classification of the additive mask.

    Returns (klass, col0) where col0[k, c] is the first tq column (multiple
    of 128) of the chunk that is not fully masked -- matmuls/exp for the
    columns before it are skipped (their softmax weights are exactly 0).
    """
    klass = np.empty((TKC, NT), dtype=np.int32)
    col0 = np.zeros((TKC, NT), dtype=np.int32)
    for k in range(TKC):
        for c in range(NT):
            blk = mask[c * 512:(c + 1) * 512, k * P:(k + 1) * P]
            mx = float(blk.max())
            mn = float(blk.min())
            if mx < -80.0:
                klass[k, c] = MSK_SKIP
                continue
            if mx == 0.0 and mn == 0.0:
                klass[k, c] = MSK_ZERO
            else:
                klass[k, c] = MSK_ADD
            # leading fully-masked tq columns, rounded down to 128
            colmax = blk.max(axis=1)          # per-tq-row max over this tile
            nz = np.nonzero(colmax >= -80.0)[0]
            first = int(nz[0]) if len(nz) else 0
            first = (first // P) * P
            # only safe to skip if every column before `first` is fully masked
            if first > 0 and float(blk[:first].max()) < -80.0:
                col0[k, c] = first
    # never allow a fully-empty (all-skip) tq chunk; keep one tile live
    for c in range(NT):
        if all(klass[k, c] == MSK_SKIP for k in range(TKC)):
            klass[min(c * 4, TKC - 1), c] = MSK_ADD
    return klass, col0


def _build_module(klass, col0, phases=(1, 2, 25, 3)):
    nc = bass.Bass()

    # inputs are pre-reblocked on the host so every DMA is contiguous
    xTb = nc.declare_dram_parameter("xTb", [NT, DIM, 512], BF16, isOutput=False)
    woT = nc.declare_dram_parameter("woT", [DIM, JPC], BF16, isOutput=False)
    woR = nc.declare_dram_parameter("woR", [JPC, DIM], BF16, isOutput=False)
    wkT = nc.declare_dram_parameter("wkT", [P, NKC * HD], BF16,
                                    isOutput=False)
    wvT = nc.declare_dram_parameter("wvT", [P, NKC * HD], BF16,
                                    isOutput=False)
    qb = nc.declare_dram_parameter("qb", [P, HPC], F32, isOutput=False)
    qb2 = nc.declare_dram_parameter("qb2", [P, HPC], F32, isOutput=False)
    kb = nc.declare_dram_parameter("kb", [P, 1], F32, isOutput=False)
    vb = nc.declare_dram_parameter("vb", [P, 1], F32, isOutput=False)
    maskTb = nc.declare_dram_parameter("maskTb", [NT, T, 512], BF16,
                                       isOutput=False)
    cost = nc.declare_dram_parameter("cost", [P, T], BF16, isOutput=False)
    sint = nc.declare_dram_parameter("sint", [P, T], BF16, isOutput=False)
    rT = nc.declare_dram_parameter("rT", [P, P], BF16, isOutput=False)
    outTb = nc.declare_dram_parameter("outTb", [NT, JPC, 512], F32,
                                      isOutput=True)

    rs_in = nc.dram_tensor("rs_in", [NT, NCORE * JPC, 512], BF16)
    rs_out = nc.dram_tensor("rs_out", [NT, JPC, 512], BF16)

    with tile.TileContext(nc) as tc:
        with (
            tc.tile_pool(name="wpool", bufs=1) as wpool,
            tc.tile_pool(name="const", bufs=1) as constp,
            tc.tile_pool(name="qkv", bufs=1) as qkvp,
            tc.tile_pool(name="qc", bufs=2) as qcp,
            tc.tile_pool(name="aout", bufs=2) as aop,
            tc.tile_pool(name="ro", bufs=2) as rop,
            tc.tile_pool(name="xs", bufs=5) as xsp,
            tc.tile_pool(name="stage", bufs=4) as stagep,
            tc.tile_pool(name="att", bufs=4) as attp,
            tc.tile_pool(name="acc", bufs=4, space="PSUM") as accp,
            tc.tile_pool(name="satt", bufs=2, space="PSUM") as sattp,
            tc.tile_pool(name="attden", bufs=2, space="PSUM") as adp,
        ):
            # ---- resident weights / tables -------------------------------
            # wo and x(0) pieces interleaved on one queue in need-time order:
            # the q-pass consumes k-chunk k at ~0.85us/chunk, so each piece
            # must land just before the PE reaches it.
            wo_s = wpool.tile([P, NKC, JPC], BF16)
            x0q = [xsp.tile([P, 8, 512], BF16, name=f"xt0_{q}", tag="xs")
                   for q in range(4)]

            def wo_piece(klo, khi):
                nc.sync.dma_start(
                    out=wo_s[:, klo:khi, :],
                    in_=woT[klo * P:khi * P, :].rearrange("(k p) j -> p k j",
                                                          p=P))

            def x0_piece(q, klo, khi):
                nc.sync.dma_start(
                    out=x0q[q][:, klo:khi, :],
                    in_=xTb[0, (8 * q + klo) * P:(8 * q + khi) * P,
                            :].rearrange("(k p) t -> p k t", p=P))

            wo_piece(0, 1)
            x0_piece(0, 0, 2)
            wo_piece(1, 2)
            x0_piece(0, 2, 8)
            wo_piece(2, 4)
            x0_piece(1, 0, 4)
            wo_piece(4, 8)
            x0_piece(1, 4, 8)
            wo_piece(8, 12)
            x0_piece(2, 0, 4)
            wo_piece(12, 16)
            x0_piece(2, 4, 8)
            wo_piece(16, 20)
            x0_piece(3, 0, 4)
            wo_piece(20, 24)
            x0_piece(3, 4, 8)
            wk_s = wpool.tile([P, NKC, HD], BF16)
            nc.sync.dma_start(out=wk_s, in_=wkT[:, :])
            wo_piece(24, 28)
            wo_piece(28, 32)
            wv_s = wpool.tile([P, NKC, HD], BF16)
            nc.sync.dma_start(out=wv_s, in_=wvT[:, :])
            # chunk-0 mask tiles: tiny, needed at ~50us, and they must not
            # queue behind the 8MiB of x(1)/wo3 traffic
            pre_mtiles = {}
            for k in range(TKC):
                if klass[k, 0] == MSK_ADD:
                    mt = attp.tile([P, 512], BF16, name=f"mt0_{k}",
                                   tag="mskpre", bufs=4)
                    nc.sync.dma_start(
                        out=mt, in_=maskTb[0, k * P:(k + 1) * P, :])
                    pre_mtiles[k] = mt

            cos_s = constp.tile([P, T], BF16)
            sin_s = constp.tile([P, T], BF16)
            nc.sync.dma_start(out=cos_s, in_=cost[:, :])
            nc.sync.dma_start(out=sin_s, in_=sint[:, :])

            rT_s = constp.tile([P, P], BF16)
            nc.sync.dma_start(out=rT_s, in_=rT[:, :])
            qb_s = constp.tile([P, HPC], F32)
            qb2_s = constp.tile([P, HPC], F32)
            kb_s = constp.tile([P, 1], F32)
            vb_s = constp.tile([P, 1], F32)
            nc.sync.dma_start(out=qb_s, in_=qb[:, :])
            nc.sync.dma_start(out=qb2_s, in_=qb2[:, :])
            nc.sync.dma_start(out=kb_s, in_=kb[:, :])
            nc.sync.dma_start(out=vb_s, in_=vb[:, :])

            # row-slice of wo for the contraction-sharded output projection
            # (loaded lazily -- per-m DMAs are emitted inside the n==0 body so
            # they don't compete with x/wo for early DMA bandwidth)
            wo3_s = wpool.tile([P, HPC, DIM], BF16)

            ones_s = constp.tile([P, P], BF16)
            nc.vector.memset(ones_s, 1.0)
            ident_s = constp.tile([P, P], BF16)
            make_identity(nc, ident_s)

            # persistent K/V in rope-d transposed layout (Q is per-chunk)
            kT_s = qkvp.tile([P, T], BF16)        # [hd, t]
            vN_s = qkvp.tile([P, TKC, HD], BF16)  # [tk%128, tk//128, hd]

            # ---- phase 1: projections for all t-chunks ------------------
            def emit_x(n):
                # x for one t-chunk: four 8-k-chunk quarter tiles
                tiles = []
                for q in range(4):
                    ks = slice(q * 8 * P, (q + 1) * 8 * P)
                    xq = xsp.tile([P, 8, 512], BF16, name=f"xt{n}_{q}",
                                  tag="xs")
                    nc.sync.dma_start(
                        out=xq,
                        in_=xTb[n, ks, :].rearrange("(k p) t -> p k t", p=P))
                    tiles.append(xq)
                return tiles

            next_xtq = None
            for n in range(NT):
                ts = slice(n * 512, (n + 1) * 512)
                xtq = next_xtq

                if n == 0:
                    xtq = x0q

                def xt_sl(k, xtq=xtq):
                    return xtq[k // 8][:, k % 8, :]

                acc_tiles = []
                for m in range(HPC + 2):  # 4 q-head tiles, k, v
                    pacc = accp.tile([P, 512], F32, name=f"pacc{n}_{m}",
                                     tag="acc")
                    acc_tiles.append(pacc)
                # q-pass
                for k in range(NKC):
                    for m in range(HPC):
                        nc.tensor.matmul(
                            acc_tiles[m],
                            lhsT=wo_s[:, k, m * P:(m + 1) * P],
                            rhs=xt_sl(k),
                            start=(k == 0),
                            stop=(k == NKC - 1),
                        )
                # q biases on Act while the kv-pass runs on PE
                braw_q = []
                for m in range(HPC):
                    braw = stagep.tile([P, 512], BF16, name=f"braw{n}_{m}",
                                       tag="braw")
                    nc.scalar.add(braw, acc_tiles[m], qb_s[:, m:m + 1])
                    braw_q.append(braw)
                # rope table slices for this chunk (small, late-need DMAs)
                nc.sync.dma_start(out=cos_s[:, ts], in_=cost[:, ts])
                nc.sync.dma_start(out=sin_s[:, ts], in_=sint[:, ts])
                # k-pass then v-pass: the k accumulator finishes at the
                # halfway point, so its bias (and the psum bank the third
                # rotation matmul reuses) is ready before the rotations
                for k in range(NKC):
                    nc.tensor.matmul(
                        acc_tiles[HPC], lhsT=wk_s[:, k, :], rhs=xt_sl(k),
                        start=(k == 0), stop=(k == NKC - 1),
                    )
                for k in range(NKC):
                    nc.tensor.matmul(
                        acc_tiles[HPC + 1], lhsT=wv_s[:, k, :], rhs=xt_sl(k),
                        start=(k == 0), stop=(k == NKC - 1),
                    )
                # issue the next chunk's x loads now (Pool queue is free of
                # collective waits during the projection phase)
                if n + 1 < NT:
                    next_xtq = emit_x(n + 1)
                if n == 0:
                    # wo row-slice for the output projection: not needed for
                    # ~150us, so loaded after the startup-critical DMAs
                    for m in range(HPC):
                        nc.sync.dma_start(out=wo3_s[:, m, :],
                                          in_=woR[m * P:(m + 1) * P, :])
                # k bias first: it frees the psum bank that the third
                # q-rotation matmul reuses; v bias next for the transposes
                brawk = stagep.tile([P, 512], BF16, name=f"brawk{n}",
                                    tag="braw")
                nc.scalar.add(brawk, acc_tiles[HPC], kb_s[:, 0:1])
                v_st = stagep.tile([P, 512], BF16, name=f"vst{n}", tag="braw")
                nc.scalar.add(v_st, acc_tiles[HPC + 1], vb_s[:, 0:1])

                # rotation matmuls for q tiles + k tile (PE, after kv-pass)
                qc_s = qcp.tile([P, HPC, 512], BF16, name=f"qc{n}", tag="qc")
                rot_q = []
                for m in range(HPC):
                    rot_ps = accp.tile([P, 512], F32, name=f"rot{n}_{m}",
                                       tag="acc")
                    nc.tensor.matmul(rot_ps, lhsT=rT_s, rhs=braw_q[m],
                                     start=True, stop=True)
                    rot_q.append(rot_ps)
                rot_k = accp.tile([P, 512], F32, name=f"rotk{n}", tag="acc")
                nc.tensor.matmul(rot_k, lhsT=rT_s, rhs=brawk,
                                 start=True, stop=True)
                # v transpose into natural layout
                for j in range(4):
                    vt_ps = accp.tile([P, P], BF16, name=f"vt{n}_{j}",
                                      tag="acc")
                    nc.tensor.transpose(vt_ps, v_st[:, j * P:(j + 1) * P],
                                        ident_s)
                    nc.scalar.copy(vN_s[:, n * 4 + j, :], vt_ps)

                # rope combine on DVE (all-bf16 for 2x mode where possible)
                for m in range(HPC):
                    dst = qc_s[:, m, :]
                    tmp = stagep.tile([P, 512], BF16, name=f"tmp{n}_{m}",
                                      tag="stage")
                    nc.vector.tensor_mul(tmp, rot_q[m], sin_s[:, ts])
                    nc.vector.tensor_mul(dst, braw_q[m], cos_s[:, ts])
                    nc.vector.tensor_add(dst, dst, tmp)
                tmpk = stagep.tile([P, 512], BF16, name=f"tmpk{n}", tag="stage")
                nc.vector.tensor_mul(tmpk, rot_k, sin_s[:, ts])
                nc.vector.tensor_mul(kT_s[:, ts], brawk, cos_s[:, ts])
                nc.vector.tensor_add(kT_s[:, ts], kT_s[:, ts], tmpk)

                # ---- attention, partial out-proj, ReduceScatter ------
                c = n
                act_ks = [k for k in range(TKC) if klass[k, c] != MSK_SKIP]
                add_ks = [k for k in act_ks if klass[k, c] == MSK_ADD]
                if c == 0:
                    mtiles = pre_mtiles
                else:
                    mtiles = {}
                    for k in add_ks:
                        mt = attp.tile([P, 512], BF16, name=f"mt{c}_{k}",
                                       tag="msk", bufs=max(2, len(add_ks) + 1))
                        nc.sync.dma_start(
                            out=mt, in_=maskTb[c, k * P:(k + 1) * P, :])
                        mtiles[k] = mt

                attn_c = aop.tile([P, HPC, 512], BF16, name=f"ac{c}", tag="ac")
                nact = len(act_ks)
                pairs = [(h, i, k) for h in range(HPC)
                         for i, k in enumerate(act_ks)]
                state = {}

                jctr = [0]

                def emit_score(h, i, k, c=c, qc_s=qc_s, mtiles=mtiles,
                               state=state, jctr=jctr):
                    off = 0 if i == 0 else int(col0[k, c])
                    # every third score borrows a psum bank from the (idle
                    # during attention) projection pool: 3 scores in flight
                    # cover the exp round-trip latency
                    j = jctr[0]
                    jctr[0] += 1
                    pool, tg = ((accp, "acc") if j % 3 == 2
                                else (sattp, "satt"))
                    s_ps = pool.tile([P, 512], F32, name=f"sps{c}_{h}_{k}",
                                     tag=tg)
                    nc.tensor.matmul(
                        s_ps[:, off:],
                        lhsT=kT_s[:, k * P:(k + 1) * P],
                        rhs=qc_s[:, h, off:],
                        start=True, stop=True,
                    )
                    if k in mtiles:
                        nc.vector.tensor_add(s_ps[:, off:], s_ps[:, off:],
                                             mtiles[k][:, off:])
                    e_sb = attp.tile([P, 512], BF16, name=f"e{c}_{h}_{k}",
                                     tag="exp", bufs=6)
                    # exp(SCALE * s + mask): mask was pre-divided by
                    # SCALE on the host, so the add can happen upstream.
                    nc.scalar.activation(
                        e_sb[:, off:], s_ps[:, off:],
                        mybir.ActivationFunctionType.Exp, scale=SCALE)
                    state[(h, i)] = (e_sb, off)

                def emit_avden(h, i, c=c, act_ks=act_ks, nact=nact,
                               state=state, attn_c=attn_c):
                    if i == 0:
                        state[h, "apv"] = adp.tile(
                            [P, 512], F32, name=f"apv{c}_{h}", tag="attden")
                        state[h, "den"] = adp.tile(
                            [P, 512], F32, name=f"den{c}_{h}", tag="attden")
                    e_sb, off = state.pop((h, i))
                    k = act_ks[i]
                    nc.tensor.matmul(
                        state[h, "apv"][:, off:], lhsT=vN_s[:, k, :],
                        rhs=e_sb[:, off:],
                        start=(i == 0), stop=(i == nact - 1),
                    )
                    nc.tensor.matmul(
                        state[h, "den"][:, off:], lhsT=ones_s,
                        rhs=e_sb[:, off:],
                        start=(i == 0), stop=(i == nact - 1),
                    )
                    if i == nact - 1:
                        rcp = attp.tile([P, 512], F32, name=f"rcp{c}_{h}",
                                        tag="rcp", bufs=2)
                        nc.vector.reciprocal(rcp, state.pop((h, "den")))
                        nc.vector.tensor_mul(attn_c[:, h, :],
                                             state.pop((h, "apv")), rcp)

                # software-pipelined emission with lookahead 3: three
                # scores sit between a pair's score and its exp-dependent
                # matmuls, covering the exp latency and (at head boundaries)
                # the DVE normalize that frees the psum accumulator slots.
                LA = 3
                for j, (h, i, k) in enumerate(pairs):
                    emit_score(h, i, k)
                    if j >= LA:
                        ph, pi, _ = pairs[j - LA]
                        emit_avden(ph, pi)
                for j in range(max(0, len(pairs) - LA), len(pairs)):
                    emit_avden(*pairs[j][:2])

                # partial output projection: contraction over this core's 512
                # attention dims, all 4096 output dims; psum banks borrowed
                # from the (idle) attention pools.  Output dims are split in
                # two halves with one ReduceScatter each so the first
                # collective fires while the second half is still on the PE.
                # (8, 24) o-tile split: the small leading collective
                # clears the device before the big piece's data is ready,
                # minimizing the exposed tail after the last chunk
                rs_splits = {7: (0, 8), 31: (8, 32)}
                for jo in range(NKC):
                    po = (sattp if jo % 2 == 0 else adp).tile(
                        [P, 512], F32, name=f"po{c}_{jo}",
                        tag="satt" if jo % 2 == 0 else "attden")
                    for m in range(HPC):
                        nc.tensor.matmul(
                            po,
                            lhsT=wo3_s[:, m, jo * P:(jo + 1) * P],
                            rhs=attn_c[:, m, :],
                            start=(m == 0),
                            stop=(m == HPC - 1),
                        )
                    osb = attp.tile([P, 512], BF16, name=f"osb{c}_{jo}",
                                    tag="osb", bufs=4)
                    # GPSIMD cannot read PSUM; alternate DVE/Act for the
                    # psum->sbuf downcast copies
                    if jo % 2 == 0:
                        nc.vector.tensor_copy(osb, po)
                    else:
                        nc.scalar.copy(osb, po)
                    nc.sync.dma_start(out=rs_in[c, jo * P:(jo + 1) * P, :],
                                      in_=osb)
                    if jo in rs_splits:
                        lo, hi = rs_splits[jo]
                        nc.gpsimd.collective_compute(
                            "ReduceScatter",
                            mybir.AluOpType.add,
                            replica_groups=[list(range(NCORE))],
                            ins=[rs_in[c, lo * P:hi * P, :]],
                            outs=[rs_out[c, lo * P // NCORE:
                                          hi * P // NCORE, :]],
                        )


            # ---- phase 4: bias + writeback (SP + DVE, after everything) --
            # core cix's rows for half hf map to output dims
            # 2048*hf + 256*cix + [0, 256); bias slices (qb2) and the
            # host-side reassembly account for this.  Emitted last so the
            # rs_out reads (which wait on the collectives) never block the
            # mask/osb DMAs on the SP queue.
            # tile_wait_until pins these to the end of every engine queue in
            # the scheduler's virtual clock: a phase-4 op waiting on a
            # collective must never head-block mask/x DMAs or rope ops.
            with tc.tile_wait_until(10.0):
                for c in range(NT):
                    ro_s = rop.tile([P, HPC, 512], BF16, name=f"ro{c}",
                                    tag="ro", bufs=2)
                    # piece A: rs_out rows [0,128) = 1 tile; piece B:
                    # rows [128,512) = 3 tiles
                    nc.sync.dma_start(out=ro_s[:, 0, :],
                                      in_=rs_out[c, 0:P, :])
                    o_sb = stagep.tile([P, 512], F32, name=f"o{c}_0",
                                       tag="ostage", bufs=2)
                    nc.vector.tensor_scalar_add(o_sb, ro_s[:, 0, :],
                                                qb2_s[:, 0:1])
                    nc.sync.dma_start(out=outTb[c, 0:P, :], in_=o_sb)
                    nc.sync.dma_start(
                        out=ro_s[:, 1:4, :],
                        in_=rs_out[c, P:JPC, :].rearrange(
                            "(m p) t -> p m t", p=P))
                    for m in range(1, HPC):
                        o_sb = stagep.tile([P, 512], F32, name=f"o{c}_{m}",
                                           tag="ostage", bufs=2)
                        nc.vector.tensor_scalar_add(o_sb, ro_s[:, m, :],
                                                    qb2_s[:, m:m + 1])
                        nc.sync.dma_start(out=outTb[c, m * P:(m + 1) * P, :],
                                          in_=o_sb)

    legalize_waits(nc)
    return nc


def _marshal_inputs(x, freqs_cos, freqs_sin, mask, wk_w, wk_b, wv_w, wv_b,
                    wo_w, wo_b):
    bf = ml_dtypes.bfloat16
    x = np.asarray(x, np.float32)
    mask = np.asarray(mask, np.float32)
    cos = np.asarray(freqs_cos, np.float32)
    sin = np.asarray(freqs_sin, np.float32)
    wk_w = np.asarray(wk_w, np.float32)
    wv_w = np.asarray(wv_w, np.float32)
    wo_w = np.asarray(wo_w, np.float32)
    wk_b = np.asarray(wk_b, np.float32)
    wv_b = np.asarray(wv_b, np.float32)
    wo_b = np.asarray(wo_b, np.float32)

    xT = x.reshape(T, DIM).T                       # (DIM, T)
    xTb = np.ascontiguousarray(
        xT.reshape(DIM, NT, 512).transpose(1, 0, 2).astype(bf))
    # mask applied on-device as exp(SCALE*s + SCALE*maskT): pre-divide, and
    # reblock (tq-chunk, tk, tq') so every mask tile DMA is contiguous
    maskT = mask.T / np.float32(SCALE)             # (tk, tq)
    maskTb = np.ascontiguousarray(
        maskT.reshape(T, NT, 512).transpose(1, 0, 2).astype(bf))

    cos2 = np.repeat(cos.T, 2, axis=0)  # (128, T): rows 2i,2i+1 = cos[:, i]
    sin2 = np.repeat(sin.T, 2, axis=0)

    # rotation matmul constant: out = R @ q with rot[2i] = -q[2i+1],
    # rot[2i+1] = q[2i]; lhsT layout (R transposed).
    RT = np.zeros((P, P), np.float32)
    idx = np.arange(0, P, 2)
    RT[idx + 1, idx] = -1.0
    RT[idx, idx + 1] = 1.0

    common = dict(
        xTb=xTb, maskTb=maskTb,
        cost=np.ascontiguousarray(cos2.astype(bf)),
        sint=np.ascontiguousarray(sin2.astype(bf)),
        rT=RT.astype(bf),
    )

    woT_full = wo_w.T  # (DIM in, DIM out): woT_full[d, o] = wo_w[o, d]
    in_maps = []
    for cix in range(NCORE):
        jlo = cix * JPC
        klo = cix * HD
        m = dict(common)
        m["woT"] = np.ascontiguousarray(wo_w[jlo:jlo + JPC, :].T.astype(bf))
        m["woR"] = np.ascontiguousarray(woT_full[jlo:jlo + JPC, :].astype(bf))
        # pre-block [DIM, HD] -> [P, NKC*HD]: partition row p holds the
        # k-chunk-major weights so one 8KB-contiguous DMA suffices
        wkTf = wk_w[klo:klo + HD, :].T.astype(bf)     # (DIM, HD)
        wvTf = wv_w[klo:klo + HD, :].T.astype(bf)
        m["wkT"] = np.ascontiguousarray(
            wkTf.reshape(NKC, P, HD).transpose(1, 0, 2).reshape(P, NKC * HD))
        m["wvT"] = np.ascontiguousarray(
            wvTf.reshape(NKC, P, HD).transpose(1, 0, 2).reshape(P, NKC * HD))
        m["qb"] = np.ascontiguousarray(wo_b[jlo:jlo + JPC].reshape(HPC, P).T)
        # phase-4 bias for the (8, 24)-o-tile ReduceScatter split:
        # row block 0 holds output dims 128*cix + [0, 128); row block
        # m in {1,2,3} holds 1024 + 384*cix + 128*(m-1) + [0, 128)
        qb2 = np.empty((P, HPC), np.float32)
        qb2[:, 0] = wo_b[128 * cix:128 * cix + P]
        for m2 in range(3):
            base = 1024 + 384 * cix + 128 * m2
            qb2[:, 1 + m2] = wo_b[base:base + P]
        m["qb2"] = np.ascontiguousarray(qb2)
        m["kb"] = np.ascontiguousarray(wk_b[klo:klo + HD].reshape(1, P).T)
        m["vb"] = np.ascontiguousarray(wv_b[klo:klo + HD].reshape(1, P).T)
        in_maps.append(m)
    return in_maps, mask


def run(inputs, trace=False):
    """Build, run on 8 cores, return (full_output, BassKernelResults)."""
    in_maps, mask = _marshal_inputs(
        inputs["x"], inputs["freqs_cos"], inputs["freqs_sin"], inputs["mask"],
        inputs["wk_w"], inputs["wk_b"], inputs["wv_w"], inputs["wv_b"],
        inputs["wo_w"], inputs["wo_b"])
    klass, col0 = _classify_mask(mask)
    nc = _build_module(klass, col0)
    res = run_bass_kernel_spmd(nc, in_maps, core_ids=list(range(NCORE)),
                               trace=trace)
    out = np.empty((DIM, T), np.float32)
    for cix in range(NCORE):
        ob = res.results[cix]["outTb"]          # (NT, JPC, 512)
        for n in range(NT):
            cols = slice(n * 512, (n + 1) * 512)
            out[128 * cix:128 * cix + P, cols] = ob[n, 0:P]
            for m2 in range(3):
                base = 1024 + 384 * cix + 128 * m2
                out[base:base + P, cols] = ob[n, (1 + m2) * P:(2 + m2) * P]
    out = out.T  # (T, DIM)
    return np.ascontiguousarray(out[None, :, :]).astype(np.float32), res


def kernel(**inputs):
    out, _ = run(inputs, trace=False)
    return out


# revision 26
# speedup vs baseline: 1.0517x; 1.0087x over previous
"""Tensor-parallel GQA attention prefill (B=1, T=2048, D=4096, 32 q-heads /
8 kv-heads) for 8 Trainium2 NeuronCores.

Sharding: head-parallel.  Core c owns q-heads [4c, 4c+4) and kv-head c.
  phase 1: Q/K/V projections in transposed layout (head-dim on partitions),
           RoPE applied via a rotation-matmul + two table multiplies.
  phase 2: per-head attention with scores held transposed (tk on
           partitions); softmax denominators come from a ones-matmul;
           fully-masked tiles are skipped (host inspects the mask tensor).
  phase 3: output projection sharded over the CONTRACTION dim: each core
           multiplies its local attention slice (512 rows) against its
           512-row slice of wo, producing partial sums for ALL 4096 output
           dims; a per-tq-chunk ReduceScatter (bf16) then both sums the
           partials and hands each core its own 512 output rows.
  phase 4: bias add + writeback of the scattered result.

Matmul operands are bf16 (fp32 accumulation in PSUM); measured end-to-end
error vs the fp32 reference is ~4e-3 relative.

NOTE: faithful to the reference "bug" -- the q projection uses wo_w/wo_b.
"""

import numpy as np
import ml_dtypes

import bass_rust
import concourse.bass as bass
import concourse.mybir as mybir
import concourse.tile as tile
from concourse.bass_utils import run_bass_kernel_spmd
from concourse.masks import make_identity

# problem constants (self-contained; do not read spec.json)
DIM = 4096
NH = 32
NKV = 8
HD = 128
T = 2048
NCORE = 8
HPC = NH // NCORE      # 4 q heads per core
JPC = HPC * HD         # 512 output columns per core
P = 128
NT = T // 512          # 4 free-dim chunks of 512
NKC = DIM // P         # 32 contraction chunks in the projections
TKC = T // P           # 16 tk chunks in attention
SCALE = 1.0 / float(np.sqrt(HD))

F32 = mybir.dt.float32
BF16 = mybir.dt.bfloat16

# mask tile classification
MSK_SKIP, MSK_ZERO, MSK_ADD = 0, 1, 2


def legalize_waits(nc, max_waits=1):
    """Hoist excess on_wait conditions onto preceding nop instructions.

    This walrus build rejects instructions carrying more than a couple of
    sync-wait commands; engines execute their queue in order, so a nop that
    waits immediately before the real instruction is equivalent.
    """
    n_new = 0
    for f in nc.m.functions:
        for bb in f.blocks:
            insts = bb.instructions
            new = []
            for ins in list(insts):
                si = ins.sync_info
                waits = list(si.on_wait) if si is not None and si.on_wait else []
                if len(waits) > max_waits:
                    hoist = waits[:-max_waits]
                    keep = waits[-max_waits:]
                    for j in range(0, len(hoist), max_waits):
                        chunk = hoist[j:j + max_waits]
                        nop = mybir.InstNoOp(
                            name=f"{ins.name}_hw{j}",
                            engine=ins.engine,
                            sync_info=bass_rust.SyncInfo(
                                on_wait=chunk, on_update=[]),
                        )
                        new.append(nop)
                        n_new += 1
                    ins.sync_info = bass_rust.SyncInfo(
                        on_wait=keep,
                        on_update=list(si.on_update) if si.on_update else [])
                new.append(ins)
            insts.clear()
            insts.extend(new)
    return n_new


def _classify_mask(mask):
    """Per (tk-chunk, tq-chunk-of-512) classification of the additive mask.

    Returns (klass, col0) where col0[k, c] is the first tq column (multiple
    of 128) of the chunk that is not fully masked -- matmuls/exp for the
    columns before it are skipped (their softmax weights are exactly 0).
    """
    klass = np.empty((TKC, NT), dtype=np.int32)
    col0 = np.zeros((TKC, NT), dtype=np.int32)
    for k in range(TKC):
        for c in range(NT):
            blk = mask[c * 512:(c + 1) * 512, k * P:(k + 1) * P]
            mx = float(blk.max())
            mn = float(blk.min())
            if mx < -80.0:
                klass[k, c] = MSK_SKIP
                continue
            if mx == 0.0 and mn == 0.0:
                klass[k, c] = MSK_ZERO
            else:
                klass[k, c] = MSK_ADD
            # leading fully-masked tq columns, rounded down to 128
            colmax = blk.max(axis=1)          # per-tq-row max over this tile
            nz = np.nonzero(colmax >= -80.0)[0]
            first = int(nz[0]) if len(nz) else 0
            first = (first // P) * P
            # only safe to skip if every column before `first` is fully masked
            if first > 0 and float(blk[:first].max()) < -80.0:
                col0[k, c] = first
    # never allow a fully-empty (all-skip) tq chunk; keep one tile live
    for c in range(NT):
        if all(klass[k, c] == MSK_SKIP for k in range(TKC)):
            klass[min(c * 4, TKC - 1), c] = MSK_ADD
    return klass, col0


def _build_module(klass, col0, phases=(1, 2, 25, 3)):
    nc = bass.Bass()

    # inputs are pre-reblocked on the host so every DMA is contiguous
    xTb = nc.declare_dram_parameter("xTb", [NT, DIM, 512], BF16, isOutput=False)
    woT = nc.declare_dram_parameter("woT", [DIM, JPC], BF16, isOutput=False)
    woR = nc.declare_dram_parameter("woR", [JPC, DIM], BF16, isOutput=False)
    wkT = nc.declare_dram_parameter("wkT", [P, NKC * HD], BF16,
                                    isOutput=False)
    wvT = nc.declare_dram_parameter("wvT", [P, NKC * HD], BF16,
                                    isOutput=False)
    qb = nc.declare_dram_parameter("qb", [P, HPC], F32, isOutput=False)
    qb2 = nc.declare_dram_parameter("qb2", [P, HPC], F32, isOutput=False)
    kb = nc.declare_dram_parameter("kb", [P, 1], F32, isOutput=False)
    vb = nc.declare_dram_parameter("vb", [P, 1], F32, isOutput=False)
    maskTb = nc.declare_dram_parameter("maskTb", [NT, T, 512], BF16,
                                       isOutput=False)
    cost = nc.declare_dram_parameter("cost", [P, T], BF16, isOutput=False)
    sint = nc.declare_dram_parameter("sint", [P, T], BF16, isOutput=False)
    rT = nc.declare_dram_parameter("rT", [P, P], BF16, isOutput=False)
    outTb = nc.declare_dram_parameter("outTb", [NT, JPC, 512], F32,
                                      isOutput=True)

    rs_in = nc.dram_tensor("rs_in", [NT, NCORE * JPC, 512], BF16)
    rs_out = nc.dram_tensor("rs_out", [NT, JPC, 512], BF16)

    with tile.TileContext(nc) as tc:
        with (
            tc.tile_pool(name="wpool", bufs=1) as wpool,
            tc.tile_pool(name="const", bufs=1) as constp,
            tc.tile_pool(name="qkv", bufs=1) as qkvp,
            tc.tile_pool(name="qc", bufs=2) as qcp,
            tc.tile_pool(name="aout", bufs=2) as aop,
            tc.tile_pool(name="ro", bufs=2) as rop,
            tc.tile_pool(name="xs", bufs=5) as xsp,
            tc.tile_pool(name="stage", bufs=4) as stagep,
            tc.tile_pool(name="att", bufs=4) as attp,
            tc.tile_pool(name="acc", bufs=4, space="PSUM") as accp,
            tc.tile_pool(name="satt", bufs=2, space="PSUM") as sattp,
            tc.tile_pool(name="attden", bufs=2, space="PSUM") as adp,
        ):
            # ---- resident weights / tables -------------------------------
            # wo and x(0) pieces interleaved on one queue in need-time order:
            # the q-pass consumes k-chunk k at ~0.85us/chunk, so each piece
            # must land just before the PE reaches it.
            wo_s = wpool.tile([P, NKC, JPC], BF16)
            x0q = [xsp.tile([P, 8, 512], BF16, name=f"xt0_{q}", tag="xs")
                   for q in range(4)]

            def wo_piece(klo, khi):
                nc.sync.dma_start(
                    out=wo_s[:, klo:khi, :],
                    in_=woT[klo * P:khi * P, :].rearrange("(k p) j -> p k j",
                                                          p=P))

            def x0_piece(q, klo, khi):
                nc.sync.dma_start(
                    out=x0q[q][:, klo:khi, :],
                    in_=xTb[0, (8 * q + klo) * P:(8 * q + khi) * P,
                            :].rearrange("(k p) t -> p k t", p=P))

            wo_piece(0, 1)
            x0_piece(0, 0, 2)
            wo_piece(1, 2)
            x0_piece(0, 2, 8)
            wo_piece(2, 4)
            x0_piece(1, 0, 4)
            wo_piece(4, 8)
            x0_piece(1, 4, 8)
            wo_piece(8, 12)
            x0_piece(2, 0, 4)
            wo_piece(12, 16)
            x0_piece(2, 4, 8)
            wo_piece(16, 20)
            x0_piece(3, 0, 4)
            wo_piece(20, 24)
            x0_piece(3, 4, 8)
            wk_s = wpool.tile([P, NKC, HD], BF16)
            nc.sync.dma_start(out=wk_s, in_=wkT[:, :])
            wo_piece(24, 28)
            wo_piece(28, 32)
            wv_s = wpool.tile([P, NKC, HD], BF16)
            nc.sync.dma_start(out=wv_s, in_=wvT[:, :])
            # chunk-0 mask tiles: tiny, needed at ~50us, and they must not
            # queue behind the 8MiB of x(1)/wo3 traffic
            pre_mtiles = {}
            for k in range(TKC):
                if klass[k, 0] == MSK_ADD:
                    mt = attp.tile([P, 512], BF16, name=f"mt0_{k}",
                                   tag="mskpre", bufs=4)
                    nc.sync.dma_start(
                        out=mt, in_=maskTb[0, k * P:(k + 1) * P, :])
                    pre_mtiles[k] = mt

            cos_s = constp.tile([P, T], BF16)
            sin_s = constp.tile([P, T], BF16)
            nc.sync.dma_start(out=cos_s, in_=cost[:, :])
            nc.sync.dma_start(out=sin_s, in_=sint[:, :])

            rT_s = constp.tile([P, P], BF16)
            nc.sync.dma_start(out=rT_s, in_=rT[:, :])
            qb_s = constp.tile([P, HPC], F32)
            qb2_s = constp.tile([P, HPC], F32)
            kb_s = constp.tile([P, 1], F32)
            vb_s = constp.tile([P, 1], F32)
            nc.sync.dma_start(out=qb_s, in_=qb[:, :])
            nc.sync.dma_start(out=qb2_s, in_=qb2[:, :])
            nc.sync.dma_start(out=kb_s, in_=kb[:, :])
            nc.sync.dma_start(out=vb_s, in_=vb[:, :])

            # row-slice of wo for the contraction-sharded output projection
            # (loaded lazily -- per-m DMAs are emitted inside the n==0 body so
            # they don't compete with x/wo for early DMA bandwidth)
            wo3_s = wpool.tile([P, HPC, DIM], BF16)

            ones_s = constp.tile([P, P], BF16)
            nc.vector.memset(ones_s, 1.0)
            ident_s = constp.tile([P, P], BF16)
            make_identity(nc, ident_s)

            # persistent K/V in rope-d transposed layout (Q is per-chunk)
            kT_s = qkvp.tile([P, T], BF16)        # [hd, t]
            vN_s = qkvp.tile([P, TKC, HD], BF16)  # [tk%128, tk//128, hd]

            # ---- phase 1: projections for all t-chunks ------------------
            def emit_x(n):
                # x for one t-chunk: four 8-k-chunk quarter tiles
                tiles = []
                for q in range(4):
                    ks = slice(q * 8 * P, (q + 1) * 8 * P)
                    xq = xsp.tile([P, 8, 512], BF16, name=f"xt{n}_{q}",
                                  tag="xs")
                    nc.sync.dma_start(
                        out=xq,
                        in_=xTb[n, ks, :].rearrange("(k p) t -> p k t", p=P))
                    tiles.append(xq)
                return tiles

            next_xtq = None
            for n in range(NT):
                ts = slice(n * 512, (n + 1) * 512)
                xtq = next_xtq

                if n == 0:
                    xtq = x0q

                def xt_sl(k, xtq=xtq):
                    return xtq[k // 8][:, k % 8, :]

                acc_tiles = []
                for m in range(HPC + 2):  # 4 q-head tiles, k, v
                    pacc = accp.tile([P, 512], F32, name=f"pacc{n}_{m}",
                                     tag="acc")
                    acc_tiles.append(pacc)
                # q-pass
                for k in range(NKC):
                    for m in range(HPC):
                        nc.tensor.matmul(
                            acc_tiles[m],
                            lhsT=wo_s[:, k, m * P:(m + 1) * P],
                            rhs=xt_sl(k),
                            start=(k == 0),
                            stop=(k == NKC - 1),
                        )
                # q biases on Act while the kv-pass runs on PE
                braw_q = []
                for m in range(HPC):
                    braw = stagep.tile([P, 512], BF16, name=f"braw{n}_{m}",
                                       tag="braw")
                    nc.scalar.add(braw, acc_tiles[m], qb_s[:, m:m + 1])
                    braw_q.append(braw)
                # rope table slices for this chunk (small, late-need DMAs)
                nc.sync.dma_start(out=cos_s[:, ts], in_=cost[:, ts])
                nc.sync.dma_start(out=sin_s[:, ts], in_=sint[:, ts])
                # k-pass then v-pass: the k accumulator finishes at the
                # halfway point, so its bias (and the psum bank the third
                # rotation matmul reuses) is ready before the rotations
                for k in range(NKC):
                    nc.tensor.matmul(
                        acc_tiles[HPC], lhsT=wk_s[:, k, :], rhs=xt_sl(k),
                        start=(k == 0), stop=(k == NKC - 1),
                    )
                for k in range(NKC):
                    nc.tensor.matmul(
                        acc_tiles[HPC + 1], lhsT=wv_s[:, k, :], rhs=xt_sl(k),
                        start=(k == 0), stop=(k == NKC - 1),
                    )
                # issue the next chunk's x loads now (Pool queue is free of
                # collective waits during the projection phase)
                if n + 1 < NT:
                    next_xtq = emit_x(n + 1)
                if n == 0:
                    # wo row-slice for the output projection: not needed for
                    # ~150us, so loaded after the startup-critical DMAs
                    for m in range(HPC):
                        nc.sync.dma_start(out=wo3_s[:, m, :],
                                          in_=woR[m * P:(m + 1) * P, :])
                # k bias first: it frees the psum bank that the third
                # q-rotation matmul reuses; v bias next for the transposes
                brawk = stagep.tile([P, 512], BF16, name=f"brawk{n}",
                                    tag="braw")
                nc.scalar.add(brawk, acc_tiles[HPC], kb_s[:, 0:1])
                v_st = stagep.tile([P, 512], BF16, name=f"vst{n}", tag="braw")
                nc.scalar.add(v_st, acc_tiles[HPC + 1], vb_s[:, 0:1])

                # rotation matmuls for q tiles + k tile (PE, after kv-pass)
                qc_s = qcp.tile([P, HPC, 512], BF16, name=f"qc{n}", tag="qc")
                rot_q = []
                for m in range(HPC):
                    rot_ps = accp.tile([P, 512], F32, name=f"rot{n}_{m}",
                                       tag="acc")
                    nc.tensor.matmul(rot_ps, lhsT=rT_s, rhs=braw_q[m],
                                     start=True, stop=True)
                    rot_q.append(rot_ps)
                rot_k = accp.tile([P, 512], F32, name=f"rotk{n}", tag="acc")
                nc.tensor.matmul(rot_k, lhsT=rT_s, rhs=brawk,
                                 start=True, stop=True)
                # v transpose into natural layout
                for j in range(4):
                    vt_ps = accp.tile([P, P], BF16, name=f"vt{n}_{j}",
                                      tag="acc")
                    nc.tensor.transpose(vt_ps, v_st[:, j * P:(j + 1) * P],
                                        ident_s)
                    nc.scalar.copy(vN_s[:, n * 4 + j, :], vt_ps)

                # rope combine on DVE (all-bf16 for 2x mode where possible)
                for m in range(HPC):
                    dst = qc_s[:, m, :]
                    tmp = stagep.tile([P, 512], BF16, name=f"tmp{n}_{m}",
                                      tag="stage")
                    nc.vector.tensor_mul(tmp, rot_q[m], sin_s[:, ts])
                    nc.vector.tensor_mul(dst, braw_q[m], cos_s[:, ts])
                    nc.vector.tensor_add(dst, dst, tmp)
                tmpk = stagep.tile([P, 512], BF16, name=f"tmpk{n}", tag="stage")
                nc.vector.tensor_mul(tmpk, rot_k, sin_s[:, ts])
                nc.vector.tensor_mul(kT_s[:, ts], brawk, cos_s[:, ts])
                nc.vector.tensor_add(kT_s[:, ts], kT_s[:, ts], tmpk)

                # ---- attention, partial out-proj, ReduceScatter ------
                c = n
                act_ks = [k for k in range(TKC) if klass[k, c] != MSK_SKIP]
                add_ks = [k for k in act_ks if klass[k, c] == MSK_ADD]
                if c == 0:
                    mtiles = pre_mtiles
                else:
                    mtiles = {}
                    for k in add_ks:
                        mt = attp.tile([P, 512], BF16, name=f"mt{c}_{k}",
                                       tag="msk", bufs=max(2, len(add_ks) + 1))
                        nc.sync.dma_start(
                            out=mt, in_=maskTb[c, k * P:(k + 1) * P, :])
                        mtiles[k] = mt

                attn_c = aop.tile([P, HPC, 512], BF16, name=f"ac{c}", tag="ac")
                nact = len(act_ks)
                pairs = [(h, i, k) for h in range(HPC)
                         for i, k in enumerate(act_ks)]
                state = {}

                jctr = [0]

                def emit_score(h, i, k, c=c, qc_s=qc_s, mtiles=mtiles,
                               state=state, jctr=jctr):
                    off = 0 if i == 0 else int(col0[k, c])
                    # every third score borrows a psum bank from the (idle
                    # during attention) projection pool: 3 scores in flight
                    # cover the exp round-trip latency
                    j = jctr[0]
                    jctr[0] += 1
                    pool, tg = ((accp, "acc") if j % 3 == 2
                                else (sattp, "satt"))
                    s_ps = pool.tile([P, 512], F32, name=f"sps{c}_{h}_{k}",
                                     tag=tg)
                    nc.tensor.matmul(
                        s_ps[:, off:],
                        lhsT=kT_s[:, k * P:(k + 1) * P],
                        rhs=qc_s[:, h, off:],
                        start=True, stop=True,
                    )
                    if k in mtiles:
                        nc.vector.tensor_add(s_ps[:, off:], s_ps[:, off:],
                                             mtiles[k][:, off:])
                    e_sb = attp.tile([P, 512], BF16, name=f"e{c}_{h}_{k}",
                                     tag="exp", bufs=6)
                    # exp(SCALE * s + mask): mask was pre-divided by
                    # SCALE on the host, so the add can happen upstream.
                    nc.scalar.activation(
                        e_sb[:, off:], s_ps[:, off:],
                        mybir.ActivationFunctionType.Exp, scale=SCALE)
                    state[(h, i)] = (e_sb, off)

                def emit_avden(h, i, c=c, act_ks=act_ks, nact=nact,
                               state=state, attn_c=attn_c):
                    if i == 0:
                        state[h, "apv"] = adp.tile(
                            [P, 512], F32, name=f"apv{c}_{h}", tag="attden")
                        state[h, "den"] = adp.tile(
                            [P, 512], F32, name=f"den{c}_{h}", tag="attden")
                    e_sb, off = state.pop((h, i))
                    k = act_ks[i]
                    nc.tensor.matmul(
                        state[h, "apv"][:, off:], lhsT=vN_s[:, k, :],
                        rhs=e_sb[:, off:],
                        start=(i == 0), stop=(i == nact - 1),
                    )
                    nc.tensor.matmul(
                        state[h, "den"][:, off:], lhsT=ones_s,
                        rhs=e_sb[:, off:],
                        start=(i == 0), stop=(i == nact - 1),
                    )
                    if i == nact - 1:
                        rcp = attp.tile([P, 512], F32, name=f"rcp{c}_{h}",
                                        tag="rcp", bufs=2)
                        nc.vector.reciprocal(rcp, state.pop((h, "den")))
                        nc.vector.tensor_mul(attn_c[:, h, :],
                                             state.pop((h, "apv")), rcp)

                # software-pipelined emission with lookahead 3: three
                # scores sit between a pair's score and its exp-dependent
                # matmuls, covering the exp latency and (at head boundaries)
                # the DVE normalize that frees the psum accumulator slots.
                LA = 3
                for j, (h, i, k) in enumerate(pairs):
                    emit_score(h, i, k)
                    if j >= LA:
                        ph, pi, _ = pairs[j - LA]
                        emit_avden(ph, pi)
                for j in range(max(0, len(pairs) - LA), len(pairs)):
                    emit_avden(*pairs[j][:2])

                # partial output projection: contraction over this core's 512
                # attention dims, all 4096 output dims; psum banks borrowed
                # from the (idle) attention pools.  Output dims are split in
                # two halves with one ReduceScatter each so the first
                # collective fires while the second half is still on the PE.
                # (8, 24) o-tile split: the small leading collective
                # clears the device before the big piece's data is ready,
                # minimizing the exposed tail after the last chunk
                rs_splits = {7: (0, 8), 31: (8, 32)}
                for jo in range(NKC):
                    po = (sattp if jo % 2 == 0 else adp).tile(
                        [P, 512], F32, name=f"po{c}_{jo}",
                        tag="satt" if jo % 2 == 0 else "attden")
                    for m in range(HPC):
                        nc.tensor.matmul(
                            po,
                            lhsT=wo3_s[:, m, jo * P:(jo + 1) * P],
                            rhs=attn_c[:, m, :],
                            start=(m == 0),
                            stop=(m == HPC - 1),
                        )
                    osb = attp.tile([P, 512], BF16, name=f"osb{c}_{jo}",
                                    tag="osb", bufs=4)
                    # GPSIMD cannot read PSUM; alternate DVE/Act for the
                    # psum->sbuf downcast copies
                    if jo % 2 == 0:
                        nc.vector.tensor_copy(osb, po)
                    else:
                        nc.scalar.copy(osb, po)
                    nc.sync.dma_start(out=rs_in[c, jo * P:(jo + 1) * P, :],
                                      in_=osb)
                    if jo in rs_splits:
                        lo, hi = rs_splits[jo]
                        nc.gpsimd.collective_compute(
                            "ReduceScatter",
                            mybir.AluOpType.add,
                            replica_groups=[list(range(NCORE))],
                            ins=[rs_in[c, lo * P:hi * P, :]],
                            outs=[rs_out[c, lo * P // NCORE:
                                          hi * P // NCORE, :]],
                        )


            # ---- phase 4: bias + writeback (SP + DVE, after everything) --
            # core cix's rows for half hf map to output dims
            # 2048*hf + 256*cix + [0, 256); bias slices (qb2) and the
            # host-side reassembly account for this.  Emitted last so the
            # rs_out reads (which wait on the collectives) never block the
            # mask/osb DMAs on the SP queue.
            # tile_wait_until pins these to the end of every engine queue in
            # the scheduler's virtual clock: a phase-4 op waiting on a
            # collective must never head-block mask/x DMAs or rope ops.
            with tc.tile_wait_until(10.0):
                for c in range(NT):
                    ro_s = rop.tile([P, HPC, 512], BF16, name=f"ro{c}",
                                    tag="ro", bufs=2)
                    # piece A: rs_out rows [0,128) = 1 tile; piece B:
                    # rows [128,512) = 3 tiles
                    nc.sync.dma_start(out=ro_s[:, 0, :],
                                      in_=rs_out[c, 0:P, :])
                    o_sb = stagep.tile([P, 512], F32, name=f"o{c}_0",
                                       tag="ostage", bufs=2)
                    nc.vector.tensor_scalar_add(o_sb, ro_s[:, 0, :],
                                                qb2_s[:, 0:1])
                    nc.sync.dma_start(out=outTb[c, 0:P, :], in_=o_sb)
                    nc.sync.dma_start(
                        out=ro_s[:, 1:4, :],
                        in_=rs_out[c, P:JPC, :].rearrange(
                            "(m p) t -> p m t", p=P))
                    # piece B: one staged tile + one DMA so the tail is not
                    # three serialized add->issue->transfer round trips
                    oB = stagep.tile([P, 3, 512], F32, name=f"oB{c}",
                                     tag="ostageB", bufs=1)
                    for m in range(1, HPC):
                        nc.vector.tensor_scalar_add(oB[:, m - 1, :],
                                                    ro_s[:, m, :],
                                                    qb2_s[:, m:m + 1])
                    nc.sync.dma_start(
                        out=outTb[c, P:JPC, :].rearrange("(m p) t -> p m t",
                                                         p=P),
                        in_=oB)

    legalize_waits(nc)
    return nc


def _marshal_inputs(x, freqs_cos, freqs_sin, mask, wk_w, wk_b, wv_w, wv_b,
                    wo_w, wo_b):
    bf = ml_dtypes.bfloat16
    x = np.asarray(x, np.float32)
    mask = np.asarray(mask, np.float32)
    cos = np.asarray(freqs_cos, np.float32)
    sin = np.asarray(freqs_sin, np.float32)
    wk_w = np.asarray(wk_w, np.float32)
    wv_w = np.asarray(wv_w, np.float32)
    wo_w = np.asarray(wo_w, np.float32)
    wk_b = np.asarray(wk_b, np.float32)
    wv_b = np.asarray(wv_b, np.float32)
    wo_b = np.asarray(wo_b, np.float32)

    xT = x.reshape(T, DIM).T                       # (DIM, T)
    xTb = np.ascontiguousarray(
        xT.reshape(DIM, NT, 512).transpose(1, 0, 2).astype(bf))
    # mask applied on-device as exp(SCALE*s + SCALE*maskT): pre-divide, and
    # reblock (tq-chunk, tk, tq') so every mask tile DMA is contiguous
    maskT = mask.T / np.float32(SCALE)             # (tk, tq)
    maskTb = np.ascontiguousarray(
        maskT.reshape(T, NT, 512).transpose(1, 0, 2).astype(bf))

    cos2 = np.repeat(cos.T, 2, axis=0)  # (128, T): rows 2i,2i+1 = cos[:, i]
    sin2 = np.repeat(sin.T, 2, axis=0)

    # rotation matmul constant: out = R @ q with rot[2i] = -q[2i+1],
    # rot[2i+1] = q[2i]; lhsT layout (R transposed).
    RT = np.zeros((P, P), np.float32)
    idx = np.arange(0, P, 2)
    RT[idx + 1, idx] = -1.0
    RT[idx, idx + 1] = 1.0

    common = dict(
        xTb=xTb, maskTb=maskTb,
        cost=np.ascontiguousarray(cos2.astype(bf)),
        sint=np.ascontiguousarray(sin2.astype(bf)),
        rT=RT.astype(bf),
    )

    woT_full = wo_w.T  # (DIM in, DIM out): woT_full[d, o] = wo_w[o, d]
    in_maps = []
    for cix in range(NCORE):
        jlo = cix * JPC
        klo = cix * HD
        m = dict(common)
        m["woT"] = np.ascontiguousarray(wo_w[jlo:jlo + JPC, :].T.astype(bf))
        m["woR"] = np.ascontiguousarray(woT_full[jlo:jlo + JPC, :].astype(bf))
        # pre-block [DIM, HD] -> [P, NKC*HD]: partition row p holds the
        # k-chunk-major weights so one 8KB-contiguous DMA suffices
        wkTf = wk_w[klo:klo + HD, :].T.astype(bf)     # (DIM, HD)
        wvTf = wv_w[klo:klo + HD, :].T.astype(bf)
        m["wkT"] = np.ascontiguousarray(
            wkTf.reshape(NKC, P, HD).transpose(1, 0, 2).reshape(P, NKC * HD))
        m["wvT"] = np.ascontiguousarray(
            wvTf.reshape(NKC, P, HD).transpose(1, 0, 2).reshape(P, NKC * HD))
        m["qb"] = np.ascontiguousarray(wo_b[jlo:jlo + JPC].reshape(HPC, P).T)
        # phase-4 bias for the (8, 24)-o-tile ReduceScatter split:
        # row block 0 holds output dims 128*cix + [0, 128); row block
        # m in {1,2,3} holds 1024 + 384*cix + 128*(m-1) + [0, 128)
        qb2 = np.empty((P, HPC), np.float32)
        qb2[:, 0] = wo_b[128 * cix:128 * cix + P]
        for m2 in range(3):
            base = 1024 + 384 * cix + 128 * m2
            qb2[:, 1 + m2] = wo_b[base:base + P]
        m["qb2"] = np.ascontiguousarray(qb2)
        m["kb"] = np.ascontiguousarray(wk_b[klo:klo + HD].reshape(1, P).T)
        m["vb"] = np.ascontiguousarray(wv_b[klo:klo + HD].reshape(1, P).T)
        in_maps.append(m)
    return in_maps, mask


def run(inputs, trace=False):
    """Build, run on 8 cores, return (full_output, BassKernelResults)."""
    in_maps, mask = _marshal_inputs(
        inputs["x"], inputs["freqs_cos"], inputs["freqs_sin"], inputs["mask"],
        inputs["wk_w"], inputs["wk_b"], inputs["wv_w"], inputs["wv_b"],
        inputs["wo_w"], inputs["wo_b"])
    klass, col0 = _classify_mask(mask)
    nc = _build_module(klass, col0)
    res = run_bass_kernel_spmd(nc, in_maps, core_ids=list(range(NCORE)),
                               trace=trace)
    out = np.empty((DIM, T), np.float32)
    for cix in range(NCORE):
        ob = res.results[cix]["outTb"]          # (NT, JPC, 512)
        for n in range(NT):
            cols = slice(n * 512, (n + 1) * 512)
            out[128 * cix:128 * cix + P, cols] = ob[n, 0:P]
            for m2 in range(3):
                base = 1024 + 384 * cix + 128 * m2
                out[base:base + P, cols] = ob[n, (1 + m2) * P:(2 + m2) * P]
    out = out.T  # (T, DIM)
    return np.ascontiguousarray(out[None, :, :]).astype(np.float32), res


def kernel(**inputs):
    out, _ = run(inputs, trace=False)
    return out
